# revision 5
# baseline (speedup 1.0000x reference)
"""Trainium2 Bass kernel for nn_CNSYN_59528246723247.

Data-parallel over batch across 8 NeuronCores (64 batches/core), no
collectives. Per core, rows (4096 set + 64 inst, padded to 33x128) stream
through one interleaved pipeline:
  - ONE indirect-DMA gather per chunk ([128, 1100] bf16) from a host-joined
    bf16 table embj[i] = [emb[i] | emb[ca32[i,0]] | ... | emb[ca32[i,9]]]
    (data-independent O(V) join; bf16 halves DMA and doubles DVE rates)
  - context aggregation: scores/alpha on DVE in bf16 (2x perf mode),
    weighted sum + transpose to feature-major via bf16 PE accumulate-matmuls
    into one [100, 256] PSUM tile (ctx|set); drains split DVE/Act
  - Q1 MLPs feature-major on PE, all-bf16 (bias folded via ones-row); the
    masked sum over S becomes PE matmuls against a host-built 0/1 mask
  - Q2 MLPs feature-major on PE with HOST-PRETRANSPOSED contiguous bf16
    weight slabs (2-4KB DMA lines); both paths emission-interleaved
Outputs are assembled on host into the reference's 4-tuple.
"""

import sys

sys.path.insert(0, "/opt/trn_rl_repo")

from contextlib import ExitStack

import numpy as np
import ml_dtypes

import concourse.bass as bass
import concourse.mybir as mybir
import concourse.tile as tile
from concourse import bacc
from concourse.bass import IndirectOffsetOnAxis
from concourse.bass_utils import run_bass_kernel_spmd

# ---------------------------------------------------------------- dimensions
B, S, C, E = 512, 64, 10, 100
V, NH, CH = 100000, 1024, 2048
CH2 = CH // 2
NCORES = 8
BC = B // NCORES            # 64 batches per core
R_REAL = BC * S + BC        # 4160 rows per core: 4096 set + 64 inst
R2 = 4224                   # 33*128, padded row count
NT = R2 // 128              # 33 row chunks everywhere

f32 = mybir.dt.float32
bf16 = mybir.dt.bfloat16
i32 = mybir.dt.int32
AF = mybir.ActivationFunctionType
ALU = mybir.AluOpType
AX = mybir.AxisListType

_CACHE = {}


# ---------------------------------------------------------------- program
def build_program():
    key = "nc"
    if key in _CACHE:
        return _CACHE[key]

    nc = bacc.Bacc("TRN2", debug=False, target_bir_lowering=False,
                   num_swdge_queues=1)

    def gather(out, in_, idx_ap):
        nc.gpsimd.indirect_dma_start(
            out=out, out_offset=None, in_=in_,
            in_offset=IndirectOffsetOnAxis(ap=idx_ap, axis=0),
        )

    # ---- DRAM parameters
    embj = nc.dram_tensor("embj", [V, E * (C + 1)], f32, kind="ExternalInput")
    ids_ch = nc.dram_tensor("ids_ch", [128, NT], i32, kind="ExternalInput")

    w1_d = nc.dram_tensor("w1", [E, E], bf16, kind="ExternalInput")
    w2b_d = nc.dram_tensor("w2b", [E + 1, NH], bf16, kind="ExternalInput")
    w1h_d = nc.dram_tensor("w1h", [E, E], bf16, kind="ExternalInput")
    w2bh_d = nc.dram_tensor("w2bh", [E + 1, NH], bf16, kind="ExternalInput")

    # Q2 weights, host-pretransposed into contiguous per-(m,k) 128x128 slabs
    q2w1_d = nc.dram_tensor("q2w1t", [128, 16 * NH], bf16, kind="ExternalInput")
    q2w2_d = nc.dram_tensor("q2w2t", [128, 8 * CH], bf16, kind="ExternalInput")
    q2hw1_d = nc.dram_tensor("q2hw1t", [128, 16 * NH], bf16,
                             kind="ExternalInput")
    q2hw2_d = nc.dram_tensor("q2hw2t", [128, 8 * CH], bf16,
                             kind="ExternalInput")

    b1c_d = nc.dram_tensor("b1c", [128, 16], f32, kind="ExternalInput")
    b2c_d = nc.dram_tensor("b2c", [128, 8], f32, kind="ExternalInput")
    w3c_d = nc.dram_tensor("w3c", [128, 8], bf16, kind="ExternalInput")
    b3_d = nc.dram_tensor("b3", [1, 1], f32, kind="ExternalInput")
    b1ch_d = nc.dram_tensor("b1ch", [128, 16], f32, kind="ExternalInput")
    b2ch_d = nc.dram_tensor("b2ch", [128, 8], f32, kind="ExternalInput")
    w3ch_d = nc.dram_tensor("w3ch", [128, 8], bf16, kind="ExternalInput")
    b3h_d = nc.dram_tensor("b3h", [1, 1], f32, kind="ExternalInput")

    gmask_d = nc.dram_tensor("gmask", [128, 64], bf16, kind="ExternalInput")
    e64_d = nc.dram_tensor("e64", [128, 64], bf16, kind="ExternalInput")
    i128_d = nc.dram_tensor("i128", [128, 128], bf16, kind="ExternalInput")
    i128f_d = nc.dram_tensor("i128f", [128, 128], f32, kind="ExternalInput")

    out_d = nc.dram_tensor("out", [2, 2 * BC], f32, kind="ExternalOutput")

    with tile.TileContext(nc) as tc, ExitStack() as ctx:
        const = ctx.enter_context(tc.tile_pool(name="const", bufs=1))
        gat = ctx.enter_context(tc.tile_pool(name="gat", bufs=4))
        work = ctx.enter_context(tc.tile_pool(name="work", bufs=4))
        big = ctx.enter_context(tc.tile_pool(name="big", bufs=1))
        y2p = ctx.enter_context(tc.tile_pool(name="y2p", bufs=2))
        qwp = ctx.enter_context(tc.tile_pool(name="qwp", bufs=4))
        q2wk = ctx.enter_context(tc.tile_pool(name="q2wk", bufs=1))

        # ---- load constants / small weights to SBUF
        ids_sb = const.tile([128, NT], i32)
        nc.sync.dma_start(ids_sb[:], ids_ch[:])
        w1_sb = const.tile([E, E], bf16)
        nc.sync.dma_start(w1_sb[:], w1_d[:])
        w2b_sb = const.tile([E + 1, NH], bf16)
        nc.sync.dma_start(w2b_sb[:], w2b_d[:])
        w1h_sb = const.tile([E, E], bf16)
        nc.sync.dma_start(w1h_sb[:], w1h_d[:])
        w2bh_sb = const.tile([E + 1, NH], bf16)
        nc.sync.dma_start(w2bh_sb[:], w2bh_d[:])
        gmask_sb = const.tile([128, 64], bf16)
        nc.sync.dma_start(gmask_sb[:], gmask_d[:])
        e64_sb = const.tile([128, 64], bf16)
        nc.sync.dma_start(e64_sb[:], e64_d[:])
        i128_sb = const.tile([128, 128], bf16)
        nc.sync.dma_start(i128_sb[:], i128_d[:])
        i128f_sb = const.tile([128, 128], f32)
        nc.sync.dma_start(i128f_sb[:], i128f_d[:])
        b1c_sb = const.tile([128, 16], f32)
        nc.sync.dma_start(b1c_sb[:], b1c_d[:])
        b2c_sb = const.tile([128, 8], f32)
        nc.sync.dma_start(b2c_sb[:], b2c_d[:])
        w3c_sb = const.tile([128, 8], bf16)
        nc.sync.dma_start(w3c_sb[:], w3c_d[:])
        b3_sb = const.tile([1, 1], f32)
        nc.sync.dma_start(b3_sb[:], b3_d[:])
        b1ch_sb = const.tile([128, 16], f32)
        nc.sync.dma_start(b1ch_sb[:], b1ch_d[:])
        b2ch_sb = const.tile([128, 8], f32)
        nc.sync.dma_start(b2ch_sb[:], b2ch_d[:])
        w3ch_sb = const.tile([128, 8], bf16)
        nc.sync.dma_start(w3ch_sb[:], w3ch_d[:])
        b3h_sb = const.tile([1, 1], f32)
        nc.sync.dma_start(b3h_sb[:], b3h_d[:])

        # xT activations, feature-major [E, R2] bf16; columns = rows
        xt_set = big.tile([E, R2], bf16)
        xt_ctx = big.tile([E, R2], bf16)

        # single shared PSUM pool; tags budget all 8 banks:
        #   qsseg 1 + qcseg 1 + xtps 2 + l2 (2x 2-bank) 4 = 8
        ps = ctx.enter_context(tc.tile_pool(name="ps", bufs=1, space="PSUM"))

        h1s = big.tile([E + 1, R2], bf16)
        h1c = big.tile([E + 1, R2], bf16)
        # engine ops need 32-aligned start partition: set rows 96..100 to
        # 1.0; the L1 relu overwrites rows 96..99 afterwards.
        nc.vector.memset(h1s[96:E + 1, :], 1.0)
        nc.vector.memset(h1c[96:E + 1, :], 1.0)
        paths = (
            ("qs", xt_set, h1s, w1_sb, w2b_sb),
            ("qc", xt_ctx, h1c, w1h_sb, w2bh_sb),
        )
        segaccs, instaccs = {}, {}
        for name, _, _, _, _ in paths:
            segaccs[name] = ps.tile([128, 512], f32, name=name + "_seg",
                                    tag=name + "seg", bufs=1)

        def emit_A(t):
            c0 = t * 128
            ec = gat.tile([128, E * (C + 1)], f32, name="ec", bufs=3)
            gather(ec[:], embj[:, :], ids_sb[:, t:t + 1])
            ent = ec[:, 0:E]
            ctx_ap = ec[:, E:E * (C + 1)]

            # scores s[p,c] = <ctx[p,c,:], ent[p,:]>; z is cancellation-
            # sensitive, so the whole score path stays fp32 (split DVE/Pool)
            CD = 6  # c-blocks on DVE; rest on Pool
            prod = work.tile([128, C * E], f32)
            nc.vector.tensor_tensor(
                prod[:, 0:CD * E].rearrange("p (c d) -> p c d", c=CD),
                ctx_ap[:, 0:CD * E].rearrange("p (c d) -> p c d", c=CD),
                ent.unsqueeze(1).to_broadcast([128, CD, E]),
                op=ALU.mult,
            )
            nc.gpsimd.tensor_tensor(
                prod[:, CD * E:].rearrange("p (c d) -> p c d", c=C - CD),
                ctx_ap[:, CD * E:].rearrange("p (c d) -> p c d", c=C - CD),
                ent.unsqueeze(1).to_broadcast([128, C - CD, E]),
                op=ALU.mult,
            )
            s_all = work.tile([128, C], f32)
            nc.vector.tensor_reduce(
                s_all[:], prod[:].rearrange("p (c d) -> p c d", c=C),
                axis=AX.X, op=ALU.add,
            )
            z = work.tile([128, 1], f32)
            nc.vector.tensor_reduce(z[:], s_all[:], axis=AX.X, op=ALU.add)
            rz = work.tile([128, 1], f32)
            nc.vector.reciprocal(rz[:], z[:])
            alpha = work.tile([128, C], f32)
            nc.vector.tensor_scalar(alpha[:], s_all[:], rz[:], None,
                                    op0=ALU.mult)
            # scaled[p, c, :] = alpha[p, c] * ctx[p, c, :]  (bf16 out: only
            # relative error, safe after normalization)
            scaled = work.tile([128, C * E], bf16)
            nc.vector.tensor_tensor(
                scaled[:, 0:CD * E].rearrange("p (c d) -> p c d", c=CD),
                ctx_ap[:, 0:CD * E].rearrange("p (c d) -> p c d", c=CD),
                alpha[:, 0:CD].unsqueeze(2).to_broadcast([128, CD, E]),
                op=ALU.mult,
            )
            nc.gpsimd.tensor_tensor(
                scaled[:, CD * E:].rearrange("p (c d) -> p c d", c=C - CD),
                ctx_ap[:, CD * E:].rearrange("p (c d) -> p c d", c=C - CD),
                alpha[:, CD:].unsqueeze(2).to_broadcast([128, C - CD, E]),
                op=ALU.mult,
            )
            # one PSUM tile holds [ctx-agg^T | ent^T] for this chunk
            xt_ps = ps.tile([E, 256], f32, tag="xtps", bufs=2)
            for c in range(C):
                nc.tensor.matmul(
                    xt_ps[:, 0:128], lhsT=scaled[:, c * E:(c + 1) * E],
                    rhs=i128_sb[:],
                    start=(c == 0), stop=(c == C - 1),
                )
            nc.tensor.matmul(xt_ps[:, 128:256], lhsT=ent, rhs=i128f_sb[:],
                             start=True, stop=True)
            nc.scalar.copy(xt_ctx[:, c0:c0 + 128], xt_ps[:, 0:128])
            nc.scalar.copy(xt_set[:, c0:c0 + 128], xt_ps[:, 128:256])

        def emit_L1(j):
            jj = j * 512
            w = min(512, R2 - jj)
            for name, xt_sb, h1, w1s, _ in paths:
                psl = ps.tile([E, 512], f32, name=name + "_l1",
                              tag="xtps", bufs=2)
                nc.tensor.matmul(psl[:, :w], lhsT=w1s[:],
                                 rhs=xt_sb[:, jj:jj + w],
                                 start=True, stop=True)
                nc.scalar.activation(h1[0:E, jj:jj + w], psl[:, :w], AF.Relu)

        def emit_L2(t):
            for name, _, h1, _, w2bs in paths:
                segacc = segaccs[name]
                yab = ps.tile([128, NH], f32, name=name + "_l2",
                              tag="l2", bufs=2)
                lhsT = h1[:, 128 * t:128 * (t + 1)]
                nc.tensor.matmul(yab[:, 0:512], lhsT=lhsT,
                                 rhs=w2bs[:, 0:512], start=True, stop=True)
                nc.tensor.matmul(yab[:, 512:NH], lhsT=lhsT,
                                 rhs=w2bs[:, 512:NH], start=True, stop=True)
                y2 = y2p.tile([128, NH], bf16, name=name + "_y2", tag="y2",
                              bufs=3)
                nc.scalar.activation(y2[:], yab[:], AF.Relu)
                if t < 32:
                    for f in range(8):
                        nc.tensor.matmul(
                            segacc[:, 64 * f + 2 * t:64 * f + 2 * t + 2],
                            lhsT=y2[:, 128 * f:128 * (f + 1)],
                            rhs=gmask_sb[:, 2 * t:2 * t + 2],
                            start=True, stop=True,
                        )
                else:
                    instacc = ps.tile([128, 512], f32, name=name + "_ins",
                                      tag="l2", bufs=2)
                    instaccs[name] = instacc
                    for f in range(8):
                        nc.tensor.matmul(
                            instacc[:, 64 * f:64 * (f + 1)],
                            lhsT=y2[:, 128 * f:128 * (f + 1)],
                            rhs=e64_sb[:],
                            start=True, stop=True,
                        )

        for t in range(NT):
            emit_A(t)
            if t % 4 == 3:
                emit_L1(t // 4)
            if t >= 4:
                emit_L2(t - 4)
        emit_L1(8)
        for t in range(NT - 4, NT):
            emit_L2(t)

        def build_x2(name):
            segacc, instacc = segaccs[name], instaccs[name]
            # q2 inputs: [embed | embed + inst-embed], feature-major blocks
            iT = q2wk.tile([128, 512], f32, name=name + "_iT")
            nc.scalar.copy(iT[:], instacc[:])
            x2 = q2wk.tile([128, NH], bf16, name=name + "_x2")
            for f in range(8):
                nc.scalar.copy(x2[:, 128 * f:128 * f + 64],
                               segacc[:, 64 * f:64 * (f + 1)])
                nc.vector.tensor_tensor(
                    x2[:, 128 * f + 64:128 * (f + 1)],
                    segacc[:, 64 * f:64 * (f + 1)],
                    iT[:, 64 * f:64 * (f + 1)],
                    op=ALU.add,
                )
            return x2

        x2_set = build_x2("qs")
        x2_ctx = build_x2("qc")

        # ---------------- phase D: Q2 MLPs
        def q2_mlp(x2, q2w1, q2w2, b1s, b2s, w3s, b3s, out_row, name):
            # generator-based so the two paths can be emitted interleaved
            hq = q2wk.tile([128, CH], bf16, name=name + "_hq")
            for m in range(16):
                wt = qwp.tile([128, NH], bf16, name="q2w1t", bufs=8)
                nc.sync.dma_start(wt[:], q2w1[:, NH * m:NH * (m + 1)])
                psq = ps.tile([128, 128], f32, name=name + "_p1",
                              tag="l2", bufs=2)
                for k in range(8):
                    nc.tensor.matmul(
                        psq[:],
                        lhsT=wt[:, 128 * k:128 * (k + 1)],
                        rhs=x2[:, 128 * k:128 * (k + 1)],
                        start=(k == 0), stop=(k == 7),
                    )
                if m % 2 == 0:
                    nc.scalar.activation(hq[:, 128 * m:128 * (m + 1)], psq[:],
                                         AF.Relu, bias=b1s[:, m:m + 1])
                else:
                    nc.vector.tensor_scalar(
                        hq[:, 128 * m:128 * (m + 1)], psq[:],
                        b1s[:, m:m + 1], 0.0, op0=ALU.add, op1=ALU.max)
                yield
            h2 = q2wk.tile([128, CH2], bf16, name=name + "_h2")
            for m in range(8):
                wt2 = qwp.tile([128, CH], bf16, name="q2w2t", bufs=4)
                nc.sync.dma_start(wt2[:], q2w2[:, CH * m:CH * (m + 1)])
                psq = ps.tile([128, 128], f32, name=name + "_p2",
                              tag="l2", bufs=2)
                for k in range(16):
                    nc.tensor.matmul(
                        psq[:],
                        lhsT=wt2[:, 128 * k:128 * (k + 1)],
                        rhs=hq[:, 128 * k:128 * (k + 1)],
                        start=(k == 0), stop=(k == 15),
                    )
                if m % 2 == 0:
                    nc.scalar.activation(h2[:, 128 * m:128 * (m + 1)], psq[:],
                                         AF.Relu, bias=b2s[:, m:m + 1])
                else:
                    nc.vector.tensor_scalar(
                        h2[:, 128 * m:128 * (m + 1)], psq[:],
                        b2s[:, m:m + 1], 0.0, op0=ALU.add, op1=ALU.max)
                yield
            ps3 = ps.tile([1, 128], f32, name=name + "_p3",
                          tag="xtps", bufs=2)
            for k in range(8):
                nc.tensor.matmul(
                    ps3[:],
                    lhsT=w3s[:, k:k + 1],
                    rhs=h2[:, 128 * k:128 * (k + 1)],
                    start=(k == 0), stop=(k == 7),
                )
            osb = q2wk.tile([1, 128], f32, name=name + "_o")
            nc.scalar.activation(osb[:], ps3[:], AF.Identity, bias=b3s[:])
            nc.sync.dma_start(out_row, osb[:])
            yield

        gens = [
            q2_mlp(x2_set, q2w1_d, q2w2_d, b1c_sb, b2c_sb, w3c_sb, b3_sb,
                   out_d[0:1, :], "q2s"),
            q2_mlp(x2_ctx, q2hw1_d, q2hw2_d, b1ch_sb, b2ch_sb, w3ch_sb,
                   b3h_sb, out_d[1:2, :], "q2h"),
        ]
        alive = list(gens)
        while alive:
            for g in list(alive):
                try:
                    next(g)
                except StopIteration:
                    alive.remove(g)

    nc.compile()
    _CACHE[key] = nc
    return nc


# ---------------------------------------------------------------- host prep
def _q2_pretranspose(w, n_m, n_k, dt):
    # [p, m*(n_k*128) + k*128 + c] = w[128k+p, 128m+c]
    kdim, mdim = w.shape
    assert kdim == n_k * 128 and mdim == n_m * 128
    wt = w.reshape(n_k, 128, n_m, 128).transpose(1, 2, 0, 3)
    return np.ascontiguousarray(wt.reshape(128, n_m * n_k * 128)).astype(dt)


def _shared_consts():
    if "consts" in _CACHE:
        return _CACHE["consts"]
    c = {
        "e64": np.eye(128, 64).astype(ml_dtypes.bfloat16),
        "i128": np.eye(128).astype(ml_dtypes.bfloat16),
        "i128f": np.eye(128, dtype=np.float32),
    }
    _CACHE["consts"] = c
    return c


def make_in_maps(inputs):
    """inputs: dict of FULL numpy arrays keyed as in setup_inputs()."""
    inp = {k: np.asarray(v) for k, v in inputs.items()}
    set_ids = inp["set_ids"].astype(np.int32)
    inst_ids = inp["inst_ids"].astype(np.int32)
    ca32 = np.ascontiguousarray(inp["contex_array"].astype(np.int32))
    emb = np.ascontiguousarray(inp["emb"].astype(np.float32))

    embj = np.empty((V, (C + 1) * E), np.float32)
    embj[:, :E] = emb
    for c in range(C):
        embj[:, (c + 1) * E:(c + 2) * E] = emb[ca32[:, c], :]
    bf = ml_dtypes.bfloat16
    shared = {
        "embj": embj,
        "w1": np.ascontiguousarray(inp["q1_w1"].astype(np.float32)).astype(bf),
        "w2b": np.ascontiguousarray(
            np.vstack([inp["q1_w2"], inp["q1_b2"][None, :]])
        ).astype(bf),
        "w1h": np.ascontiguousarray(
            inp["q1h_w1"].astype(np.float32)).astype(bf),
        "w2bh": np.ascontiguousarray(
            np.vstack([inp["q1h_w2"], inp["q1h_b2"][None, :]])
        ).astype(bf),
        "q2w1t": _q2_pretranspose(np.asarray(inp["q2_w1"], np.float32),
                                  16, 8, bf),
        "q2w2t": _q2_pretranspose(np.asarray(inp["q2_w2"], np.float32),
                                  8, 16, bf),
        "q2hw1t": _q2_pretranspose(np.asarray(inp["q2h_w1"], np.float32),
                                   16, 8, bf),
        "q2hw2t": _q2_pretranspose(np.asarray(inp["q2h_w2"], np.float32),
                                   8, 16, bf),
        "b1c": np.ascontiguousarray(
            inp["q2_b1"].astype(np.float32).reshape(16, 128).T),
        "b2c": np.ascontiguousarray(
            inp["q2_b2"].astype(np.float32).reshape(8, 128).T),
        "w3c": np.ascontiguousarray(
            inp["q2_w3"].astype(np.float32).reshape(8, 128).T).astype(bf),
        "b3": inp["q2_b3"].astype(np.float32).reshape(1, 1),
        "b1ch": np.ascontiguousarray(
            inp["q2h_b1"].astype(np.float32).reshape(16, 128).T),
        "b2ch": np.ascontiguousarray(
            inp["q2h_b2"].astype(np.float32).reshape(8, 128).T),
        "w3ch": np.ascontiguousarray(
            inp["q2h_w3"].astype(np.float32).reshape(8, 128).T).astype(bf),
        "b3h": inp["q2h_b3"].astype(np.float32).reshape(1, 1),
    }
    shared.update(_shared_consts())

    in_maps = []
    for c in range(NCORES):
        sid = set_ids[c * BC:(c + 1) * BC]          # [64, 64]
        iid = inst_ids[c * BC:(c + 1) * BC, 0]      # [64]
        ids_flat = np.concatenate(
            [sid.reshape(-1), iid,
             np.ones(R2 - R_REAL, np.int32)]).astype(np.int32)
        ids_ch = np.ascontiguousarray(ids_flat.reshape(NT, 128).T)
        mask = (sid != 0).astype(np.float32)        # [64, 64]
        gmask = np.zeros((128, 64), np.float32)  # cast to bf16 below
        for t in range(32):
            gmask[0:64, 2 * t] = mask[2 * t, :]
            gmask[64:128, 2 * t + 1] = mask[2 * t + 1, :]
        m = dict(shared)
        m["ids_ch"] = ids_ch
        m["gmask"] = gmask.astype(ml_dtypes.bfloat16)
        in_maps.append(m)
    return in_maps


def assemble_outputs(results):
    """results: list (per core) of dicts with 'out' [2, 128]."""
    setQ2 = np.zeros((B, 1), np.float32)
    setInst = np.zeros((B, 1), np.float32)
    ctxHat = np.zeros((B, 1), np.float32)
    ctxInstHat = np.zeros((B, 1), np.float32)
    for c in range(NCORES):
        o = np.asarray(results[c]["out"])
        setQ2[c * BC:(c + 1) * BC, 0] = o[0, 0:BC]
        setInst[c * BC:(c + 1) * BC, 0] = o[0, BC:2 * BC]
        ctxHat[c * BC:(c + 1) * BC, 0] = o[1, 0:BC]
        ctxInstHat[c * BC:(c + 1) * BC, 0] = o[1, BC:2 * BC]
    return (setQ2, setInst, ctxHat, ctxInstHat)


def run_cores(inputs, trace=False, **kw):
    nc = build_program()
    in_maps = make_in_maps(inputs)
    res = run_bass_kernel_spmd(nc, in_maps, list(range(NCORES)),
                               trace=trace, **kw)
    return assemble_outputs(res.results), res


def kernel(**inputs):
    outs, _ = run_cores(inputs, trace=False)
    return outs


# revision 24
# speedup vs baseline: 1.1910x; 1.1910x over previous
"""Trainium2 Bass kernel for nn_CNSYN_59528246723247.

Data-parallel over batch across 8 NeuronCores (64 batches/core), no
collectives. Per core, rows (4096 set + 64 inst, padded to 33x128) stream
through one interleaved pipeline:
  - ONE indirect-DMA gather per chunk ([128, 1100] bf16) from a host-joined
    bf16 table embj[i] = [emb[i] | emb[ca32[i,0]] | ... | emb[ca32[i,9]]]
    (data-independent O(V) join; bf16 halves DMA and doubles DVE rates)
  - context aggregation: scores/alpha on DVE in bf16 (2x perf mode),
    weighted sum + transpose to feature-major via bf16 PE accumulate-matmuls
    into one [100, 256] PSUM tile (ctx|set); drains split DVE/Act
  - Q1 MLPs feature-major on PE, all-bf16 (bias folded via ones-row); the
    masked sum over S becomes PE matmuls against a host-built 0/1 mask
  - Q2 MLPs feature-major on PE with HOST-PRETRANSPOSED contiguous bf16
    weight slabs (2-4KB DMA lines); both paths emission-interleaved
Outputs are assembled on host into the reference's 4-tuple.
"""

import sys

sys.path.insert(0, "/opt/trn_rl_repo")

from contextlib import ExitStack

import numpy as np
import ml_dtypes

import concourse.bass as bass
import concourse.mybir as mybir
import concourse.tile as tile
from concourse import bacc
from concourse.bass import IndirectOffsetOnAxis
from concourse.bass_utils import run_bass_kernel_spmd

# ---------------------------------------------------------------- dimensions
B, S, C, E = 512, 64, 10, 100
V, NH, CH = 100000, 1024, 2048
CH2 = CH // 2
NCORES = 8
BC = B // NCORES            # 64 batches per core
R_REAL = BC * S + BC        # 4160 rows per core: 4096 set + 64 inst
R2 = 4224                   # 33*128, padded row count
NT = R2 // 128              # 33 row chunks everywhere

f32 = mybir.dt.float32
bf16 = mybir.dt.bfloat16
fp8 = mybir.dt.float8e4
i32 = mybir.dt.int32
AF = mybir.ActivationFunctionType
ALU = mybir.AluOpType
AX = mybir.AxisListType

_CACHE = {}


# ---------------------------------------------------------------- program
def build_program():
    key = "nc"
    if key in _CACHE:
        return _CACHE[key]

    nc = bacc.Bacc("TRN2", debug=False, target_bir_lowering=False,
                   num_swdge_queues=1)

    def gather(out, in_, idx_ap):
        nc.gpsimd.indirect_dma_start(
            out=out, out_offset=None, in_=in_,
            in_offset=IndirectOffsetOnAxis(ap=idx_ap, axis=0),
        )

    # ---- DRAM parameters
    embj = nc.dram_tensor("embj", [V, E * (C + 1)], f32, kind="ExternalInput")
    ids_ch = nc.dram_tensor("ids_ch", [128, NT], i32, kind="ExternalInput")

    w1_d = nc.dram_tensor("w1", [E, E], bf16, kind="ExternalInput")
    w2b_d = nc.dram_tensor("w2b", [E + 1, NH], bf16, kind="ExternalInput")
    w1h_d = nc.dram_tensor("w1h", [E, E], bf16, kind="ExternalInput")
    w2bh_d = nc.dram_tensor("w2bh", [E + 1, NH], bf16, kind="ExternalInput")

    # Q2 weights, host-pretransposed into contiguous per-(m,k) 128x128 slabs
    q2w1_d = nc.dram_tensor("q2w1t", [128, 16 * NH], bf16,
                            kind="ExternalInput")
    q2w2_d = nc.dram_tensor("q2w2t", [128, 8 * CH], bf16,
                            kind="ExternalInput")
    q2hw1_d = nc.dram_tensor("q2hw1t", [128, 16 * NH], bf16,
                             kind="ExternalInput")
    q2hw2_d = nc.dram_tensor("q2hw2t", [128, 8 * CH], bf16,
                             kind="ExternalInput")

    b1c_d = nc.dram_tensor("b1c", [1, CH], bf16, kind="ExternalInput")
    b2c_d = nc.dram_tensor("b2c", [1, CH2], bf16, kind="ExternalInput")
    w3c_d = nc.dram_tensor("w3c", [128, 8], bf16, kind="ExternalInput")
    b3_d = nc.dram_tensor("b3", [1, 1], f32, kind="ExternalInput")
    b1ch_d = nc.dram_tensor("b1ch", [1, CH], bf16, kind="ExternalInput")
    b2ch_d = nc.dram_tensor("b2ch", [1, CH2], bf16, kind="ExternalInput")
    w3ch_d = nc.dram_tensor("w3ch", [128, 8], bf16, kind="ExternalInput")
    b3h_d = nc.dram_tensor("b3h", [1, 1], f32, kind="ExternalInput")

    gmask_d = nc.dram_tensor("gmask", [128, 64], bf16, kind="ExternalInput")
    e64_d = nc.dram_tensor("e64", [128, 64], bf16, kind="ExternalInput")
    i128_d = nc.dram_tensor("i128", [128, 128], bf16, kind="ExternalInput")
    i128f_d = nc.dram_tensor("i128f", [128, 128], f32, kind="ExternalInput")

    out_d = nc.dram_tensor("out", [2, 2 * BC], f32, kind="ExternalOutput")
    zgate_d = nc.dram_tensor("zgate", [1, NT], f32, kind="ExternalOutput")

    with tile.TileContext(nc) as tc, ExitStack() as ctx:
        const = ctx.enter_context(tc.tile_pool(name="const", bufs=1))
        gat = ctx.enter_context(tc.tile_pool(name="gat", bufs=4))
        work = ctx.enter_context(tc.tile_pool(name="work", bufs=2))
        big = ctx.enter_context(tc.tile_pool(name="big", bufs=1))
        y2p = ctx.enter_context(tc.tile_pool(name="y2p", bufs=2))
        qwp = ctx.enter_context(tc.tile_pool(name="qwp", bufs=2))
        q2wk = ctx.enter_context(tc.tile_pool(name="q2wk", bufs=1))

        # ---- load constants / small weights to SBUF (ids first; the first
        # gathers are emitted from the main loop right after the pools exist)
        ids_sb = const.tile([128, NT], i32)
        nc.sync.dma_start(ids_sb[:], ids_ch[:])
        w1_sb = const.tile([E, E], bf16)
        nc.sync.dma_start(w1_sb[:], w1_d[:])
        w2b_sb = const.tile([E + 1, NH], bf16)
        nc.sync.dma_start(w2b_sb[:], w2b_d[:])
        w1h_sb = const.tile([E, E], bf16)
        nc.sync.dma_start(w1h_sb[:], w1h_d[:])
        w2bh_sb = const.tile([E + 1, NH], bf16)
        nc.sync.dma_start(w2bh_sb[:], w2bh_d[:])
        gmask_sb = const.tile([128, 64], bf16)
        nc.sync.dma_start(gmask_sb[:], gmask_d[:])
        e64_sb = const.tile([128, 64], bf16)
        nc.sync.dma_start(e64_sb[:], e64_d[:])
        i128_sb = const.tile([128, 128], bf16)
        nc.sync.dma_start(i128_sb[:], i128_d[:])
        i128f_sb = const.tile([128, 128], f32)
        nc.sync.dma_start(i128f_sb[:], i128f_d[:])
        b1c_sb = const.tile([1, CH], bf16)
        nc.sync.dma_start(b1c_sb[:], b1c_d[:])
        b2c_sb = const.tile([1, CH2], bf16)
        nc.sync.dma_start(b2c_sb[:], b2c_d[:])
        w3c_sb = const.tile([128, 8], bf16)
        nc.sync.dma_start(w3c_sb[:], w3c_d[:])
        b3_sb = const.tile([1, 1], f32)
        nc.sync.dma_start(b3_sb[:], b3_d[:])
        b1ch_sb = const.tile([1, CH], bf16)
        nc.sync.dma_start(b1ch_sb[:], b1ch_d[:])
        b2ch_sb = const.tile([1, CH2], bf16)
        nc.sync.dma_start(b2ch_sb[:], b2ch_d[:])
        w3ch_sb = const.tile([128, 8], bf16)
        nc.sync.dma_start(w3ch_sb[:], w3ch_d[:])
        b3h_sb = const.tile([1, 1], f32)
        nc.sync.dma_start(b3h_sb[:], b3h_d[:])

        # Q2 layer-1 weights become SBUF-resident during the chunk phase;
        # layer-2 head slabs (first 4 m-blocks per path) too. The prefetch
        # DMAs are emitted inside the chunk loop, paced by a tiny gate DMA
        # that reads each chunk's gathered tile (keeps the shared DMA device
        # from starving the gathers).
        # all-bf16: fp8 anywhere on the value path costs ~3% (random-sign
        # dot products don't average quantization noise down)
        pdt = {"qs": bf16, "qc": bf16}
        w1t_res = {"qs": const.tile([128, 16 * NH], bf16, name="w1t_qs"),
                   "qc": const.tile([128, 16 * NH], bf16, name="w1t_qc")}
        w2t_res = {"qs": const.tile([128, 8 * CH], bf16, name="w2t_qs"),
                   "qc": const.tile([128, 8 * CH], bf16, name="w2t_qc")}
        w1t_dram = {"qs": q2w1_d, "qc": q2hw1_d}
        w2t_dram = {"qs": q2w2_d, "qc": q2hw2_d}
        slab_jobs = []
        for m in range(16):
            for pn in ("qs", "qc"):
                slab_jobs.append((w1t_res[pn][:, NH * m:NH * (m + 1)],
                                  w1t_dram[pn][:, NH * m:NH * (m + 1)]))
        for m in range(8):
            for pn in ("qs", "qc"):
                slab_jobs.append((w2t_res[pn][:, CH * m:CH * (m + 1)],
                                  w2t_dram[pn][:, CH * m:CH * (m + 1)]))
        ones1 = {}
        for pn, dt_ in (("qs", bf16), ("qc", bf16)):
            o = const.tile([1, 128], dt_, name="ones_" + pn)
            nc.vector.memset(o[:], 1.0)
            ones1[pn] = o

        # xT activations, feature-major, interleaved per chunk as
        # [ctx 128 | set 128] so one Act instr drains both PSUM transposes.
        # Ring buffer: L1 consumes within ~6 chunks, keep an 8-chunk window.
        xt_all = big.tile([E, 8 * 256], bf16)
        xt_v = xt_all[:].rearrange("e (t x c) -> e t x c", t=8, x=2)

        # single shared PSUM pool; tags budget all 8 banks:
        #   qsseg 1 + qcseg 1 + xtps 2 + l2 (2x 2-bank) 4 = 8
        ps = ctx.enter_context(tc.tile_pool(name="ps", bufs=1, space="PSUM"))

        # h1 rings: L2 lags L1 by <8 chunks, keep two 512-col L1 blocks
        h1s = big.tile([E + 1, 1024], bf16)
        h1c = big.tile([E + 1, 1024], bf16)
        # engine ops need 32-aligned start partition: set rows 96..100 to
        # 1.0; the L1 relu overwrites rows 96..99 afterwards.
        nc.gpsimd.memset(h1s[96:E + 1, :], 1.0)
        nc.gpsimd.memset(h1c[96:E + 1, :], 1.0)
        paths = (
            ("qs", 1, h1s, w1_sb, w2b_sb),
            ("qc", 0, h1c, w1h_sb, w2bh_sb),
        )
        segaccs, instaccs = {}, {}
        for name, _, _, _, _ in paths:
            segaccs[name] = ps.tile([128, 512], f32, name=name + "_seg",
                                    tag=name + "seg", bufs=1)

        ec_tiles, scaled_tiles = {}, {}

        def emit_gather(t):
            ec = gat.tile([128, E * (C + 1)], f32, name="ec", bufs=4)
            gather(ec[:], embj[:, :], ids_sb[:, t:t + 1])
            ec_tiles[t] = ec

        def emit_scores(t):
            ec = ec_tiles[t]
            ent = ec[:, 0:E]
            ctx_ap = ec[:, E:E * (C + 1)]

            # scores s[p,c] = <ctx[p,c,:], ent[p,:]>; z is cancellation-
            # sensitive, so the whole score path stays fp32 (split DVE/Pool)
            PD = 6
            prod = work.tile([128, C * E], f32)
            nc.vector.tensor_tensor(
                prod[:, 0:PD * E].rearrange("p (c d) -> p c d", c=PD),
                ctx_ap[:, 0:PD * E].rearrange("p (c d) -> p c d", c=PD),
                ent.unsqueeze(1).to_broadcast([128, PD, E]),
                op=ALU.mult,
            )
            nc.gpsimd.tensor_tensor(
                prod[:, PD * E:].rearrange("p (c d) -> p c d", c=C - PD),
                ctx_ap[:, PD * E:].rearrange("p (c d) -> p c d", c=C - PD),
                ent.unsqueeze(1).to_broadcast([128, C - PD, E]),
                op=ALU.mult,
            )
            s_all = work.tile([128, C], f32)
            nc.vector.tensor_reduce(
                s_all[:], prod[:].rearrange("p (c d) -> p c d", c=C),
                axis=AX.X, op=ALU.add,
            )
            z = work.tile([128, 1], f32)
            nc.vector.tensor_reduce(z[:], s_all[:], axis=AX.X, op=ALU.add)
            rz = work.tile([128, 1], f32)
            nc.vector.reciprocal(rz[:], z[:])
            alpha = work.tile([128, C], f32)
            nc.vector.tensor_scalar(alpha[:], s_all[:], rz[:], None,
                                    op0=ALU.mult)
            # scaled[p, c, :] = alpha[p, c] * ctx[p, c, :]  (bf16 out: only
            # relative error, safe after normalization). 2 blocks on DVE,
            # 8 on Pool to balance engine load.
            CD = 6
            scaled = work.tile([128, C * E], bf16)
            nc.vector.tensor_tensor(
                scaled[:, 0:CD * E].rearrange("p (c d) -> p c d", c=CD),
                ctx_ap[:, 0:CD * E].rearrange("p (c d) -> p c d", c=CD),
                alpha[:, 0:CD].unsqueeze(2).to_broadcast([128, CD, E]),
                op=ALU.mult,
            )
            nc.gpsimd.tensor_tensor(
                scaled[:, CD * E:].rearrange("p (c d) -> p c d", c=C - CD),
                ctx_ap[:, CD * E:].rearrange("p (c d) -> p c d", c=C - CD),
                alpha[:, CD:].unsqueeze(2).to_broadcast([128, C - CD, E]),
                op=ALU.mult,
            )
            scaled_tiles[t] = scaled

        def emit_transposes(t):
            ec = ec_tiles.pop(t)
            ent = ec[:, 0:E]
            scaled = scaled_tiles.pop(t)
            # one PSUM tile holds [ctx-agg^T | ent^T] for this chunk
            xt_ps = ps.tile([E, 256], f32, tag="xtps", bufs=2)
            for c in range(C):
                nc.tensor.matmul(
                    xt_ps[:, 0:128], lhsT=scaled[:, c * E:(c + 1) * E],
                    rhs=i128_sb[:],
                    start=(c == 0), stop=(c == C - 1),
                )
            nc.tensor.matmul(xt_ps[:, 128:256], lhsT=ent, rhs=i128f_sb[:],
                             start=True, stop=True)
            nc.scalar.copy(
                xt_all[:, 256 * (t % 8):256 * (t % 8 + 1)], xt_ps[:])

        def emit_L1(j):
            jj = j * 512
            w = min(512, R2 - jj)
            nch = w // 128
            s0 = (4 * j) % 8
            hj = 512 * (j % 2)
            for name, xsel, h1, w1s, _ in paths:
                psl = ps.tile([E, 512], f32, name=name + "_l1",
                              tag="l2", bufs=2)
                nc.tensor.matmul(psl[:, :w], lhsT=w1s[:],
                                 rhs=xt_v[:, s0:s0 + nch, xsel, :],
                                 start=True, stop=True)
                nc.scalar.activation(h1[0:E, hj:hj + w], psl[:, :w], AF.Relu)

        def emit_L2(t):
            for name, _, h1, _, w2bs in paths:
                segacc = segaccs[name]
                yab = ps.tile([128, NH], f32, name=name + "_l2",
                              tag="l2", bufs=2)
                lhsT = h1[:, 128 * (t % 8):128 * (t % 8 + 1)]
                nc.tensor.matmul(yab[:, 0:512], lhsT=lhsT,
                                 rhs=w2bs[:, 0:512], start=True, stop=True)
                nc.tensor.matmul(yab[:, 512:NH], lhsT=lhsT,
                                 rhs=w2bs[:, 512:NH], start=True, stop=True)
                y2 = y2p.tile([128, NH], bf16, name=name + "_y2", tag="y2",
                              bufs=2)
                if t >= NT - 6 and name == "qc":
                    nc.vector.tensor_scalar(y2[:], yab[:], 0.0, None,
                                            op0=ALU.max)
                else:
                    nc.scalar.activation(y2[:], yab[:], AF.Relu)
                if t < 32:
                    for f in range(8):
                        nc.tensor.matmul(
                            segacc[:, 64 * f + 2 * t:64 * f + 2 * t + 2],
                            lhsT=y2[:, 128 * f:128 * (f + 1)],
                            rhs=gmask_sb[:, 2 * t:2 * t + 2],
                            start=True, stop=True,
                        )
                else:
                    instacc = ps.tile([128, 512], f32, name=name + "_ins",
                                      tag="l2", bufs=2)
                    instaccs[name] = instacc
                    for f in range(8):
                        nc.tensor.matmul(
                            instacc[:, 64 * f:64 * (f + 1)],
                            lhsT=y2[:, 128 * f:128 * (f + 1)],
                            rhs=e64_sb[:],
                            start=True, stop=True,
                        )

        emit_gather(0)
        emit_gather(1)
        emit_gather(2)
        n_slab = 0
        for t in range(NT):
            if t >= 6:
                emit_L2(t - 6)
            if t % 4 == 1 and t >= 5:
                emit_L1((t - 5) // 4)
            if t + 3 < NT:
                emit_gather(t + 3)
            emit_scores(t)
            # paced Q2-weight prefetch: gate on this chunk's gathered tile,
            # then ship slabs on the idle SP queue
            if 1 <= t <= 32:
                nc.sync.dma_start(zgate_d[:, t:t + 1],
                                  ec_tiles[min(t + 2, NT - 1)][0:1, 0:1])
                target = min(32, (8 * t) // 7)
                while n_slab < target:
                    dst, srcap = slab_jobs[n_slab]
                    nc.sync.dma_start(dst, srcap)
                    n_slab += 1
            if t >= 1:
                emit_transposes(t - 1)
        emit_transposes(NT - 1)
        while n_slab < len(slab_jobs):
            dst, srcap = slab_jobs[n_slab]
            nc.sync.dma_start(dst, srcap)
            n_slab += 1
        emit_L1(7)
        emit_L1(8)
        for t in range(NT - 6, NT):
            emit_L2(t)

        def build_x2(name):
            segacc, instacc = segaccs[name], instaccs[name]
            # q2 inputs: [embed | embed + inst-embed], feature-major blocks
            iT = q2wk.tile([128, 512], bf16, name=name + "_iT")
            nc.scalar.copy(iT[:], instacc[:])
            x2 = q2wk.tile([128, NH], pdt[name], name=name + "_x2")
            for f in range(8):
                nc.scalar.copy(x2[:, 128 * f:128 * f + 64],
                               segacc[:, 64 * f:64 * (f + 1)])
                nc.vector.tensor_tensor(
                    x2[:, 128 * f + 64:128 * (f + 1)],
                    segacc[:, 64 * f:64 * (f + 1)],
                    iT[:, 64 * f:64 * (f + 1)],
                    op=ALU.add,
                )
            return x2

        x2_set = build_x2("qs")
        x2_ctx = build_x2("qc")

        # ---------------- phase D: Q2 MLPs
        def q2_mlp(x2, name, b1row, b2row, w3s, b3s, out_row):
            # generator-based so the two paths can be emitted interleaved;
            # 4 m-blocks share one [128,512] PSUM tile so PE runs ~1.7us
            # bursts per drain; biases enter as a K=1 matmul of ones
            w1t_sb = w1t_res[name]
            w2t_sb = w2t_res[name]
            hq = q2wk.tile([128, CH], pdt[name], name=name + "_hq")
            for mg in range(4):
                psq = ps.tile([128, 512], f32, name=name + "_p1",
                              tag="l2", bufs=2)
                for mi in range(4):
                    m = 4 * mg + mi
                    for k in range(8):
                        nc.tensor.matmul(
                            psq[:, 128 * mi:128 * (mi + 1)],
                            lhsT=w1t_sb[:, NH * m + 128 * k:
                                        NH * m + 128 * (k + 1)],
                            rhs=x2[:, 128 * k:128 * (k + 1)],
                            start=(k == 0), stop=False,
                        )
                    nc.tensor.matmul(
                        psq[:, 128 * mi:128 * (mi + 1)],
                        lhsT=b1row[:, 128 * m:128 * (m + 1)],
                        rhs=ones1[name][:], start=False, stop=True,
                    )
                hsc = 1.0
                if mg % 2 == 0:
                    nc.scalar.activation(hq[:, 512 * mg:512 * (mg + 1)],
                                         psq[:], AF.Relu, scale=hsc)
                else:
                    nc.vector.tensor_scalar(hq[:, 512 * mg:512 * (mg + 1)],
                                            psq[:], hsc, 0.0,
                                            op0=ALU.mult, op1=ALU.max)
                yield
            h2 = q2wk.tile([128, CH2], bf16, name=name + "_h2")
            for mg in range(2):
                psq = ps.tile([128, 512], f32, name=name + "_p2",
                              tag="l2", bufs=2)
                for mi in range(4):
                    m = 4 * mg + mi
                    for k in range(16):
                        nc.tensor.matmul(
                            psq[:, 128 * mi:128 * (mi + 1)],
                            lhsT=w2t_sb[:, CH * m + 128 * k:
                                        CH * m + 128 * (k + 1)],
                            rhs=hq[:, 128 * k:128 * (k + 1)],
                            start=(k == 0), stop=False,
                        )
                    nc.tensor.matmul(
                        psq[:, 128 * mi:128 * (mi + 1)],
                        lhsT=b2row[:, 128 * m:128 * (m + 1)],
                        rhs=ones1[name][:], start=False, stop=True,
                    )
                # undo the activation scaling (qs: 2048x, qc: 512x)
                usc = 1.0 / 256
                if mg % 2 == 0:
                    nc.vector.tensor_scalar(h2[:, 512 * mg:512 * (mg + 1)],
                                            psq[:], usc, 0.0,
                                            op0=ALU.mult, op1=ALU.max)
                else:
                    nc.scalar.activation(h2[:, 512 * mg:512 * (mg + 1)],
                                         psq[:], AF.Relu, scale=usc)
                yield
            ps3 = ps.tile([1, 128], f32, name=name + "_p3",
                          tag="xtps", bufs=2)
            for k in range(8):
                nc.tensor.matmul(
                    ps3[:],
                    lhsT=w3s[:, k:k + 1],
                    rhs=h2[:, 128 * k:128 * (k + 1)],
                    start=(k == 0), stop=(k == 7),
                )
            osb = q2wk.tile([1, 128], f32, name=name + "_o")
            nc.scalar.activation(osb[:], ps3[:], AF.Identity, bias=b3s[:])
            nc.sync.dma_start(out_row, osb[:])
            yield

        gens = [
            q2_mlp(x2_set, "qs", b1c_sb, b2c_sb, w3c_sb, b3_sb,
                   out_d[0:1, :]),
            q2_mlp(x2_ctx, "qc", b1ch_sb, b2ch_sb, w3ch_sb, b3h_sb,
                   out_d[1:2, :]),
        ]
        alive = list(gens)
        while alive:
            for g in list(alive):
                try:
                    next(g)
                except StopIteration:
                    alive.remove(g)

    nc.compile()
    _CACHE[key] = nc
    return nc


# ---------------------------------------------------------------- host prep
def _q2_pretranspose(w, n_m, n_k, dt):
    # [p, m*(n_k*128) + k*128 + c] = w[128k+p, 128m+c]
    kdim, mdim = w.shape
    assert kdim == n_k * 128 and mdim == n_m * 128
    wt = w.reshape(n_k, 128, n_m, 128).transpose(1, 2, 0, 3)
    return np.ascontiguousarray(wt.reshape(128, n_m * n_k * 128)).astype(dt)


def _shared_consts():
    if "consts" in _CACHE:
        return _CACHE["consts"]
    c = {
        "e64": (np.eye(128, 64) * 256.0).astype(ml_dtypes.bfloat16),
        "i128": np.eye(128).astype(ml_dtypes.bfloat16),
        "i128f": np.eye(128, dtype=np.float32),
    }
    _CACHE["consts"] = c
    return c


def make_in_maps(inputs):
    """inputs: dict of FULL numpy arrays keyed as in setup_inputs()."""
    inp = {k: np.asarray(v) for k, v in inputs.items()}
    set_ids = inp["set_ids"].astype(np.int32)
    inst_ids = inp["inst_ids"].astype(np.int32)
    ca32 = np.ascontiguousarray(inp["contex_array"].astype(np.int32))
    emb = np.ascontiguousarray(inp["emb"].astype(np.float32))

    embj = np.empty((V, (C + 1) * E), np.float32)
    embj[:, :E] = emb
    for c in range(C):
        embj[:, (c + 1) * E:(c + 2) * E] = emb[ca32[:, c], :]
    bf = ml_dtypes.bfloat16
    f8 = ml_dtypes.float8_e4m3
    shared = {
        "embj": embj,
        "w1": np.ascontiguousarray(inp["q1_w1"].astype(np.float32)).astype(bf),
        "w2b": np.ascontiguousarray(
            np.vstack([inp["q1_w2"], inp["q1_b2"][None, :]])
        ).astype(bf),
        "w1h": np.ascontiguousarray(
            inp["q1h_w1"].astype(np.float32)).astype(bf),
        "w2bh": np.ascontiguousarray(
            np.vstack([inp["q1h_w2"], inp["q1h_b2"][None, :]])
        ).astype(bf),
        "q2w1t": _q2_pretranspose(np.asarray(inp["q2_w1"], np.float32),
                                  16, 8, bf),
        "q2w2t": _q2_pretranspose(np.asarray(inp["q2_w2"], np.float32),
                                  8, 16, bf),
        "q2hw1t": _q2_pretranspose(np.asarray(inp["q2h_w1"], np.float32),
                                   16, 8, bf),
        "q2hw2t": _q2_pretranspose(np.asarray(inp["q2h_w2"], np.float32),
                                   8, 16, bf),
        "b1c": (inp["q2_b1"].astype(np.float32) * 256).reshape(1, CH)
            .astype(bf),
        "b2c": (inp["q2_b2"].astype(np.float32) * 256).reshape(1, CH2)
            .astype(bf),
        "w3c": np.ascontiguousarray(
            inp["q2_w3"].astype(np.float32).reshape(8, 128).T).astype(bf),
        "b3": inp["q2_b3"].astype(np.float32).reshape(1, 1),
        "b1ch": (inp["q2h_b1"].astype(np.float32) * 256).reshape(1, CH)
            .astype(bf),
        "b2ch": (inp["q2h_b2"].astype(np.float32) * 256).reshape(1, CH2)
            .astype(bf),
        "w3ch": np.ascontiguousarray(
            inp["q2h_w3"].astype(np.float32).reshape(8, 128).T).astype(bf),
        "b3h": inp["q2h_b3"].astype(np.float32).reshape(1, 1),
    }
    shared.update(_shared_consts())

    in_maps = []
    for c in range(NCORES):
        sid = set_ids[c * BC:(c + 1) * BC]          # [64, 64]
        iid = inst_ids[c * BC:(c + 1) * BC, 0]      # [64]
        ids_flat = np.concatenate(
            [sid.reshape(-1), iid,
             np.ones(R2 - R_REAL, np.int32)]).astype(np.int32)
        ids_ch = np.ascontiguousarray(ids_flat.reshape(NT, 128).T)
        mask = (sid != 0).astype(np.float32) * 256.0  # [64, 64]; x256 puts
        # the fp8-cast Q2 inputs in e4m3's normal range (e4m3 max is 240)
        gmask = np.zeros((128, 64), np.float32)  # cast to bf16 below
        for t in range(32):
            gmask[0:64, 2 * t] = mask[2 * t, :]
            gmask[64:128, 2 * t + 1] = mask[2 * t + 1, :]
        m = dict(shared)
        m["ids_ch"] = ids_ch
        m["gmask"] = gmask.astype(ml_dtypes.bfloat16)
        in_maps.append(m)
    return in_maps


def assemble_outputs(results):
    """results: list (per core) of dicts with 'out' [2, 128]."""
    setQ2 = np.zeros((B, 1), np.float32)
    setInst = np.zeros((B, 1), np.float32)
    ctxHat = np.zeros((B, 1), np.float32)
    ctxInstHat = np.zeros((B, 1), np.float32)
    for c in range(NCORES):
        o = np.asarray(results[c]["out"])
        setQ2[c * BC:(c + 1) * BC, 0] = o[0, 0:BC]
        setInst[c * BC:(c + 1) * BC, 0] = o[0, BC:2 * BC]
        ctxHat[c * BC:(c + 1) * BC, 0] = o[1, 0:BC]
        ctxInstHat[c * BC:(c + 1) * BC, 0] = o[1, BC:2 * BC]
    return (setQ2, setInst, ctxHat, ctxInstHat)


def run_cores(inputs, trace=False, **kw):
    nc = build_program()
    in_maps = make_in_maps(inputs)
    res = run_bass_kernel_spmd(nc, in_maps, list(range(NCORES)),
                               trace=trace, **kw)
    return assemble_outputs(res.results), res


def kernel(**inputs):
    outs, _ = run_cores(inputs, trace=False)
    return outs


# revision 40
# speedup vs baseline: 1.1921x; 1.0009x over previous
"""Trainium2 Bass kernel for nn_CNSYN_59528246723247.

Data-parallel over batch across 8 NeuronCores (64 batches/core), no
collectives. Per core, rows (4096 set + 64 inst, padded to 33x128) stream
through one interleaved pipeline:
  - ONE indirect-DMA gather per chunk ([128, 1100] bf16) from a host-joined
    bf16 table embj[i] = [emb[i] | emb[ca32[i,0]] | ... | emb[ca32[i,9]]]
    (data-independent O(V) join; bf16 halves DMA and doubles DVE rates)
  - context aggregation: scores/alpha on DVE in bf16 (2x perf mode),
    weighted sum + transpose to feature-major via bf16 PE accumulate-matmuls
    into one [100, 256] PSUM tile (ctx|set); drains split DVE/Act
  - Q1 MLPs feature-major on PE, all-bf16 (bias folded via ones-row); the
    masked sum over S becomes PE matmuls against a host-built 0/1 mask
  - Q2 MLPs feature-major on PE with HOST-PRETRANSPOSED contiguous bf16
    weight slabs (2-4KB DMA lines); both paths emission-interleaved
Outputs are assembled on host into the reference's 4-tuple.
"""

import sys

sys.path.insert(0, "/opt/trn_rl_repo")

from contextlib import ExitStack

import numpy as np
import ml_dtypes

import concourse.bass as bass
import concourse.mybir as mybir
import concourse.tile as tile
from concourse import bacc
from concourse.bass import IndirectOffsetOnAxis
from concourse.bass_utils import run_bass_kernel_spmd

# ---------------------------------------------------------------- dimensions
B, S, C, E = 512, 64, 10, 100
V, NH, CH = 100000, 1024, 2048
CH2 = CH // 2
NCORES = 8
BC = B // NCORES            # 64 batches per core
R_REAL = BC * S + BC        # 4160 rows per core: 4096 set + 64 inst
R2 = 4224                   # 33*128, padded row count
NT = R2 // 128              # 33 row chunks everywhere

f32 = mybir.dt.float32
bf16 = mybir.dt.bfloat16
fp8 = mybir.dt.float8e4
i32 = mybir.dt.int32
AF = mybir.ActivationFunctionType
ALU = mybir.AluOpType
AX = mybir.AxisListType

_CACHE = {}


# ---------------------------------------------------------------- program
def build_program():
    key = "nc"
    if key in _CACHE:
        return _CACHE[key]

    nc = bacc.Bacc("TRN2", debug=False, target_bir_lowering=False,
                   num_swdge_queues=1)

    def gather(out, in_, idx_ap):
        nc.gpsimd.indirect_dma_start(
            out=out, out_offset=None, in_=in_,
            in_offset=IndirectOffsetOnAxis(ap=idx_ap, axis=0),
        )

    # ---- DRAM parameters
    embj = nc.dram_tensor("embj", [V, E * (C + 1)], f32, kind="ExternalInput")
    ids_ch = nc.dram_tensor("ids_ch", [128, NT], i32, kind="ExternalInput")

    w1_d = nc.dram_tensor("w1", [E, E], bf16, kind="ExternalInput")
    w2b_d = nc.dram_tensor("w2b", [E + 1, NH], bf16, kind="ExternalInput")
    w1h_d = nc.dram_tensor("w1h", [E, E], bf16, kind="ExternalInput")
    w2bh_d = nc.dram_tensor("w2bh", [E + 1, NH], bf16, kind="ExternalInput")

    # Q2 weights, host-pretransposed into contiguous per-(m,k) 128x128 slabs
    q2w1_d = nc.dram_tensor("q2w1t", [128, 16 * NH], bf16,
                            kind="ExternalInput")
    q2w2_d = nc.dram_tensor("q2w2t", [128, 8 * CH], bf16,
                            kind="ExternalInput")
    q2hw1_d = nc.dram_tensor("q2hw1t", [128, 16 * NH], bf16,
                             kind="ExternalInput")
    q2hw2_d = nc.dram_tensor("q2hw2t", [128, 8 * CH], bf16,
                             kind="ExternalInput")

    b1c_d = nc.dram_tensor("b1c", [1, CH], bf16, kind="ExternalInput")
    b2c_d = nc.dram_tensor("b2c", [1, CH2], bf16, kind="ExternalInput")
    w3c_d = nc.dram_tensor("w3c", [128, 8], bf16, kind="ExternalInput")
    b3_d = nc.dram_tensor("b3", [1, 1], f32, kind="ExternalInput")
    b1ch_d = nc.dram_tensor("b1ch", [1, CH], bf16, kind="ExternalInput")
    b2ch_d = nc.dram_tensor("b2ch", [1, CH2], bf16, kind="ExternalInput")
    w3ch_d = nc.dram_tensor("w3ch", [128, 8], bf16, kind="ExternalInput")
    b3h_d = nc.dram_tensor("b3h", [1, 1], f32, kind="ExternalInput")

    gmask_d = nc.dram_tensor("gmask", [128, 64], bf16, kind="ExternalInput")
    e64_d = nc.dram_tensor("e64", [128, 64], bf16, kind="ExternalInput")
    i128_d = nc.dram_tensor("i128", [128, 128], bf16, kind="ExternalInput")
    i128f_d = nc.dram_tensor("i128f", [128, 128], f32, kind="ExternalInput")

    out_d = nc.dram_tensor("out", [2, 2 * BC], f32, kind="ExternalOutput")
    zgate_d = nc.dram_tensor("zgate", [1, NT], f32, kind="ExternalOutput")

    with tile.TileContext(nc) as tc, ExitStack() as ctx:
        const = ctx.enter_context(tc.tile_pool(name="const", bufs=1))
        gat = ctx.enter_context(tc.tile_pool(name="gat", bufs=4))
        work = ctx.enter_context(tc.tile_pool(name="work", bufs=2))
        big = ctx.enter_context(tc.tile_pool(name="big", bufs=1))
        y2p = ctx.enter_context(tc.tile_pool(name="y2p", bufs=2))
        qwp = ctx.enter_context(tc.tile_pool(name="qwp", bufs=2))
        q2wk = ctx.enter_context(tc.tile_pool(name="q2wk", bufs=1))

        # ---- load constants / small weights to SBUF (ids first; the first
        # gathers are emitted from the main loop right after the pools exist)
        ids_sb = const.tile([128, NT], i32)
        nc.sync.dma_start(ids_sb[:], ids_ch[:])
        w1_sb = const.tile([E, E], bf16)
        nc.sync.dma_start(w1_sb[:], w1_d[:])
        w2b_sb = const.tile([E + 1, NH], bf16)
        nc.sync.dma_start(w2b_sb[:], w2b_d[:])
        w1h_sb = const.tile([E, E], bf16)
        nc.sync.dma_start(w1h_sb[:], w1h_d[:])
        w2bh_sb = const.tile([E + 1, NH], bf16)
        nc.sync.dma_start(w2bh_sb[:], w2bh_d[:])
        gmask_sb = const.tile([128, 64], bf16)
        nc.sync.dma_start(gmask_sb[:], gmask_d[:])
        e64_sb = const.tile([128, 64], bf16)
        nc.sync.dma_start(e64_sb[:], e64_d[:])
        i128_sb = const.tile([128, 128], bf16)
        nc.sync.dma_start(i128_sb[:], i128_d[:])
        i128f_sb = const.tile([128, 128], f32)
        nc.sync.dma_start(i128f_sb[:], i128f_d[:])
        b1c_sb = const.tile([1, CH], bf16)
        nc.sync.dma_start(b1c_sb[:], b1c_d[:])
        b2c_sb = const.tile([1, CH2], bf16)
        nc.sync.dma_start(b2c_sb[:], b2c_d[:])
        w3c_sb = const.tile([128, 8], bf16)
        nc.sync.dma_start(w3c_sb[:], w3c_d[:])
        b3_sb = const.tile([1, 1], f32)
        nc.sync.dma_start(b3_sb[:], b3_d[:])
        b1ch_sb = const.tile([1, CH], bf16)
        nc.sync.dma_start(b1ch_sb[:], b1ch_d[:])
        b2ch_sb = const.tile([1, CH2], bf16)
        nc.sync.dma_start(b2ch_sb[:], b2ch_d[:])
        w3ch_sb = const.tile([128, 8], bf16)
        nc.sync.dma_start(w3ch_sb[:], w3ch_d[:])
        b3h_sb = const.tile([1, 1], f32)
        nc.sync.dma_start(b3h_sb[:], b3h_d[:])

        # Q2 layer-1 weights become SBUF-resident during the chunk phase;
        # layer-2 head slabs (first 4 m-blocks per path) too. The prefetch
        # DMAs are emitted inside the chunk loop, paced by a tiny gate DMA
        # that reads each chunk's gathered tile (keeps the shared DMA device
        # from starving the gathers).
        # all-bf16: fp8 anywhere on the value path costs ~3% (random-sign
        # dot products don't average quantization noise down)
        pdt = {"qs": bf16, "qc": bf16}
        w1t_res = {"qs": const.tile([128, 16 * NH], bf16, name="w1t_qs"),
                   "qc": const.tile([128, 16 * NH], bf16, name="w1t_qc")}
        w2t_res = {"qs": const.tile([128, 8 * CH], bf16, name="w2t_qs"),
                   "qc": const.tile([128, 8 * CH], bf16, name="w2t_qc")}
        w1t_dram = {"qs": q2w1_d, "qc": q2hw1_d}
        w2t_dram = {"qs": q2w2_d, "qc": q2hw2_d}
        slab_jobs = []
        for m in range(16):
            for pn in ("qs", "qc"):
                slab_jobs.append((w1t_res[pn][:, NH * m:NH * (m + 1)],
                                  w1t_dram[pn][:, NH * m:NH * (m + 1)]))
        for m in range(8):
            for pn in ("qs", "qc"):
                slab_jobs.append((w2t_res[pn][:, CH * m:CH * (m + 1)],
                                  w2t_dram[pn][:, CH * m:CH * (m + 1)]))
        ones1 = {}
        for pn, dt_ in (("qs", bf16), ("qc", bf16)):
            o = const.tile([1, 128], dt_, name="ones_" + pn)
            nc.vector.memset(o[:], 1.0)
            ones1[pn] = o

        # xT activations, feature-major, interleaved per chunk as
        # [ctx 128 | set 128] so one Act instr drains both PSUM transposes.
        # Ring buffer: L1 consumes within ~6 chunks, keep an 8-chunk window.
        xt_all = big.tile([E, 8 * 256], bf16)
        xt_v = xt_all[:].rearrange("e (t x c) -> e t x c", t=8, x=2)

        # single shared PSUM pool; tags budget all 8 banks:
        #   qsseg 1 + qcseg 1 + xtps 2 + l2 (2x 2-bank) 4 = 8
        ps = ctx.enter_context(tc.tile_pool(name="ps", bufs=1, space="PSUM"))

        # h1 rings: L2 lags L1 by <8 chunks, keep two 512-col L1 blocks
        h1s = big.tile([E + 1, 1024], bf16)
        h1c = big.tile([E + 1, 1024], bf16)
        # engine ops need 32-aligned start partition: set rows 96..100 to
        # 1.0; the L1 relu overwrites rows 96..99 afterwards.
        nc.gpsimd.memset(h1s[96:E + 1, :], 1.0)
        nc.gpsimd.memset(h1c[96:E + 1, :], 1.0)
        paths = (
            ("qs", 1, h1s, w1_sb, w2b_sb),
            ("qc", 0, h1c, w1h_sb, w2bh_sb),
        )
        segaccs, instaccs = {}, {}
        for name, _, _, _, _ in paths:
            segaccs[name] = ps.tile([128, 512], f32, name=name + "_seg",
                                    tag=name + "seg", bufs=1)

        ec_tiles, scaled_tiles = {}, {}

        def emit_gather(t):
            ec = gat.tile([128, E * (C + 1)], f32, name="ec", bufs=4)
            gather(ec[:], embj[:, :], ids_sb[:, t:t + 1])
            ec_tiles[t] = ec

        def emit_scores(t):
            ec = ec_tiles[t]
            ent = ec[:, 0:E]
            ctx_ap = ec[:, E:E * (C + 1)]

            # scores s[p,c] = <ctx[p,c,:], ent[p,:]>; z is cancellation-
            # sensitive, so the whole score path stays fp32 (split DVE/Pool)
            PD = 6
            prod = work.tile([128, C * E], f32)
            nc.vector.tensor_tensor(
                prod[:, 0:PD * E].rearrange("p (c d) -> p c d", c=PD),
                ctx_ap[:, 0:PD * E].rearrange("p (c d) -> p c d", c=PD),
                ent.unsqueeze(1).to_broadcast([128, PD, E]),
                op=ALU.mult,
            )
            nc.gpsimd.tensor_tensor(
                prod[:, PD * E:].rearrange("p (c d) -> p c d", c=C - PD),
                ctx_ap[:, PD * E:].rearrange("p (c d) -> p c d", c=C - PD),
                ent.unsqueeze(1).to_broadcast([128, C - PD, E]),
                op=ALU.mult,
            )
            s_all = work.tile([128, C], f32)
            nc.vector.tensor_reduce(
                s_all[:], prod[:].rearrange("p (c d) -> p c d", c=C),
                axis=AX.X, op=ALU.add,
            )
            z = work.tile([128, 1], f32)
            nc.vector.tensor_reduce(z[:], s_all[:], axis=AX.X, op=ALU.add)
            rz = work.tile([128, 1], f32)
            nc.vector.reciprocal(rz[:], z[:])
            alpha = work.tile([128, C], f32)
            nc.vector.tensor_scalar(alpha[:], s_all[:], rz[:], None,
                                    op0=ALU.mult)
            # scaled[p, c, :] = alpha[p, c] * ctx[p, c, :]  (bf16 out: only
            # relative error, safe after normalization). 2 blocks on DVE,
            # 8 on Pool to balance engine load.
            CD = 6
            scaled = work.tile([128, C * E], bf16)
            nc.vector.tensor_tensor(
                scaled[:, 0:CD * E].rearrange("p (c d) -> p c d", c=CD),
                ctx_ap[:, 0:CD * E].rearrange("p (c d) -> p c d", c=CD),
                alpha[:, 0:CD].unsqueeze(2).to_broadcast([128, CD, E]),
                op=ALU.mult,
            )
            nc.gpsimd.tensor_tensor(
                scaled[:, CD * E:].rearrange("p (c d) -> p c d", c=C - CD),
                ctx_ap[:, CD * E:].rearrange("p (c d) -> p c d", c=C - CD),
                alpha[:, CD:].unsqueeze(2).to_broadcast([128, C - CD, E]),
                op=ALU.mult,
            )
            scaled_tiles[t] = scaled

        def emit_transposes(t):
            ec = ec_tiles.pop(t)
            ent = ec[:, 0:E]
            scaled = scaled_tiles.pop(t)
            # one PSUM tile holds [ctx-agg^T | ent^T] for this chunk
            xt_ps = ps.tile([E, 256], f32, tag="xtps", bufs=2)
            for c in range(C):
                nc.tensor.matmul(
                    xt_ps[:, 0:128], lhsT=scaled[:, c * E:(c + 1) * E],
                    rhs=i128_sb[:],
                    start=(c == 0), stop=(c == C - 1),
                )
            nc.tensor.matmul(xt_ps[:, 128:256], lhsT=ent, rhs=i128f_sb[:],
                             start=True, stop=True)
            nc.scalar.copy(
                xt_all[:, 256 * (t % 8):256 * (t % 8 + 1)], xt_ps[:])

        def emit_L1(j):
            jj = j * 512
            w = min(512, R2 - jj)
            nch = w // 128
            s0 = (4 * j) % 8
            hj = 512 * (j % 2)
            for name, xsel, h1, w1s, _ in paths:
                psl = ps.tile([E, 512], f32, name=name + "_l1",
                              tag="l2", bufs=2)
                nc.tensor.matmul(psl[:, :w], lhsT=w1s[:],
                                 rhs=xt_v[:, s0:s0 + nch, xsel, :],
                                 start=True, stop=True)
                nc.scalar.activation(h1[0:E, hj:hj + w], psl[:, :w], AF.Relu)

        def emit_L2(t):
            for name, _, h1, _, w2bs in paths:
                segacc = segaccs[name]
                yab = ps.tile([128, NH], f32, name=name + "_l2",
                              tag="l2", bufs=2)
                lhsT = h1[:, 128 * (t % 8):128 * (t % 8 + 1)]
                nc.tensor.matmul(yab[:, 0:512], lhsT=lhsT,
                                 rhs=w2bs[:, 0:512], start=True, stop=True)
                nc.tensor.matmul(yab[:, 512:NH], lhsT=lhsT,
                                 rhs=w2bs[:, 512:NH], start=True, stop=True)
                y2 = y2p.tile([128, NH], bf16, name=name + "_y2", tag="y2",
                              bufs=2)
                if t >= NT - 6 and name == "qc":
                    nc.vector.tensor_scalar(y2[:], yab[:], 0.0, None,
                                            op0=ALU.max)
                else:
                    nc.scalar.activation(y2[:], yab[:], AF.Relu)
                if t < 32:
                    for f in range(8):
                        nc.tensor.matmul(
                            segacc[:, 64 * f + 2 * t:64 * f + 2 * t + 2],
                            lhsT=y2[:, 128 * f:128 * (f + 1)],
                            rhs=gmask_sb[:, 2 * t:2 * t + 2],
                            start=True, stop=True,
                        )
                else:
                    instacc = ps.tile([128, 512], f32, name=name + "_ins",
                                      tag="l2", bufs=2)
                    instaccs[name] = instacc
                    for f in range(8):
                        nc.tensor.matmul(
                            instacc[:, 64 * f:64 * (f + 1)],
                            lhsT=y2[:, 128 * f:128 * (f + 1)],
                            rhs=e64_sb[:],
                            start=True, stop=True,
                        )

        emit_gather(0)
        emit_gather(1)
        emit_gather(2)
        n_slab = 0
        for t in range(NT):
            if t % 4 == 1 and t >= 5:
                emit_L1((t - 5) // 4)
            if t >= 5:
                emit_L2(t - 5)
            if t + 3 < NT:
                emit_gather(t + 3)
            emit_scores(t)
            # paced Q2-weight prefetch: gate on this chunk's gathered tile,
            # then ship slabs on the idle SP queue
            if 1 <= t <= 32:
                nc.sync.dma_start(zgate_d[:, t:t + 1],
                                  ec_tiles[min(t + 2, NT - 1)][0:1, 0:1])
                target = min(len(slab_jobs), (8 * t) // 5)
                while n_slab < target:
                    dst, srcap = slab_jobs[n_slab]
                    nc.sync.dma_start(dst, srcap)
                    n_slab += 1
            if t >= 1:
                emit_transposes(t - 1)
        emit_transposes(NT - 1)
        while n_slab < len(slab_jobs):
            dst, srcap = slab_jobs[n_slab]
            nc.sync.dma_start(dst, srcap)
            n_slab += 1
        emit_L1(7)
        emit_L1(8)
        for t in range(NT - 5, NT):
            emit_L2(t)

        def build_x2(name):
            segacc, instacc = segaccs[name], instaccs[name]
            # q2 inputs: [embed | embed + inst-embed], feature-major blocks
            iT = q2wk.tile([128, 512], bf16, name=name + "_iT")
            nc.scalar.copy(iT[:], instacc[:])
            x2 = q2wk.tile([128, NH], pdt[name], name=name + "_x2")
            for f in range(8):
                nc.scalar.copy(x2[:, 128 * f:128 * f + 64],
                               segacc[:, 64 * f:64 * (f + 1)])
                nc.vector.tensor_tensor(
                    x2[:, 128 * f + 64:128 * (f + 1)],
                    segacc[:, 64 * f:64 * (f + 1)],
                    iT[:, 64 * f:64 * (f + 1)],
                    op=ALU.add,
                )
            return x2

        x2_set = build_x2("qs")
        x2_ctx = build_x2("qc")

        # ---------------- phase D: Q2 MLPs
        def q2_mlp(x2, name, b1row, b2row, w3s, b3s, out_row):
            # generator-based so the two paths can be emitted interleaved;
            # 4 m-blocks share one [128,512] PSUM tile so PE runs ~1.7us
            # bursts per drain; biases enter as a K=1 matmul of ones
            w1t_sb = w1t_res[name]
            w2t_sb = w2t_res[name]
            hq = q2wk.tile([128, CH], pdt[name], name=name + "_hq")
            for mg in range(4):
                psq = ps.tile([128, 512], f32, name=name + "_p1",
                              tag="l2", bufs=2)
                for mi in range(4):
                    m = 4 * mg + mi
                    for k in range(8):
                        nc.tensor.matmul(
                            psq[:, 128 * mi:128 * (mi + 1)],
                            lhsT=w1t_sb[:, NH * m + 128 * k:
                                        NH * m + 128 * (k + 1)],
                            rhs=x2[:, 128 * k:128 * (k + 1)],
                            start=(k == 0), stop=False,
                        )
                    nc.tensor.matmul(
                        psq[:, 128 * mi:128 * (mi + 1)],
                        lhsT=b1row[:, 128 * m:128 * (m + 1)],
                        rhs=ones1[name][:], start=False, stop=True,
                    )
                hsc = 1.0
                if mg % 2 == 0:
                    nc.scalar.activation(hq[:, 512 * mg:512 * (mg + 1)],
                                         psq[:], AF.Relu, scale=hsc)
                else:
                    nc.vector.tensor_scalar(hq[:, 512 * mg:512 * (mg + 1)],
                                            psq[:], hsc, 0.0,
                                            op0=ALU.mult, op1=ALU.max)
                yield
            h2 = q2wk.tile([128, CH2], bf16, name=name + "_h2")
            for mg in range(2):
                psq = ps.tile([128, 512], f32, name=name + "_p2",
                              tag="l2", bufs=2)
                for mi in range(4):
                    m = 4 * mg + mi
                    for k in range(16):
                        nc.tensor.matmul(
                            psq[:, 128 * mi:128 * (mi + 1)],
                            lhsT=w2t_sb[:, CH * m + 128 * k:
                                        CH * m + 128 * (k + 1)],
                            rhs=hq[:, 128 * k:128 * (k + 1)],
                            start=(k == 0), stop=False,
                        )
                    nc.tensor.matmul(
                        psq[:, 128 * mi:128 * (mi + 1)],
                        lhsT=b2row[:, 128 * m:128 * (m + 1)],
                        rhs=ones1[name][:], start=False, stop=True,
                    )
                # undo the activation scaling (qs: 2048x, qc: 512x)
                usc = 1.0 / 256
                if mg % 2 == 0:
                    nc.vector.tensor_scalar(h2[:, 512 * mg:512 * (mg + 1)],
                                            psq[:], usc, 0.0,
                                            op0=ALU.mult, op1=ALU.max)
                else:
                    nc.scalar.activation(h2[:, 512 * mg:512 * (mg + 1)],
                                         psq[:], AF.Relu, scale=usc)
                yield
            ps3 = ps.tile([1, 128], f32, name=name + "_p3",
                          tag="xtps", bufs=2)
            for k in range(8):
                nc.tensor.matmul(
                    ps3[:],
                    lhsT=w3s[:, k:k + 1],
                    rhs=h2[:, 128 * k:128 * (k + 1)],
                    start=(k == 0), stop=(k == 7),
                )
            osb = q2wk.tile([1, 128], f32, name=name + "_o")
            nc.scalar.activation(osb[:], ps3[:], AF.Identity, bias=b3s[:])
            nc.sync.dma_start(out_row, osb[:])
            yield

        gens = [
            q2_mlp(x2_set, "qs", b1c_sb, b2c_sb, w3c_sb, b3_sb,
                   out_d[0:1, :]),
            q2_mlp(x2_ctx, "qc", b1ch_sb, b2ch_sb, w3ch_sb, b3h_sb,
                   out_d[1:2, :]),
        ]
        alive = list(gens)
        while alive:
            for g in list(alive):
                try:
                    next(g)
                except StopIteration:
                    alive.remove(g)

    nc.compile()
    _CACHE[key] = nc
    return nc


# ---------------------------------------------------------------- host prep
def _q2_pretranspose(w, n_m, n_k, dt):
    # [p, m*(n_k*128) + k*128 + c] = w[128k+p, 128m+c]
    kdim, mdim = w.shape
    assert kdim == n_k * 128 and mdim == n_m * 128
    wt = w.reshape(n_k, 128, n_m, 128).transpose(1, 2, 0, 3)
    return np.ascontiguousarray(wt.reshape(128, n_m * n_k * 128)).astype(dt)


def _shared_consts():
    if "consts" in _CACHE:
        return _CACHE["consts"]
    c = {
        "e64": (np.eye(128, 64) * 256.0).astype(ml_dtypes.bfloat16),
        "i128": np.eye(128).astype(ml_dtypes.bfloat16),
        "i128f": np.eye(128, dtype=np.float32),
    }
    _CACHE["consts"] = c
    return c


def make_in_maps(inputs):
    """inputs: dict of FULL numpy arrays keyed as in setup_inputs()."""
    inp = {k: np.asarray(v) for k, v in inputs.items()}
    set_ids = inp["set_ids"].astype(np.int32)
    inst_ids = inp["inst_ids"].astype(np.int32)
    ca32 = np.ascontiguousarray(inp["contex_array"].astype(np.int32))
    emb = np.ascontiguousarray(inp["emb"].astype(np.float32))

    embj = np.empty((V, (C + 1) * E), np.float32)
    embj[:, :E] = emb
    for c in range(C):
        embj[:, (c + 1) * E:(c + 2) * E] = emb[ca32[:, c], :]
    bf = ml_dtypes.bfloat16
    f8 = ml_dtypes.float8_e4m3
    shared = {
        "embj": embj,
        "w1": np.ascontiguousarray(inp["q1_w1"].astype(np.float32)).astype(bf),
        "w2b": np.ascontiguousarray(
            np.vstack([inp["q1_w2"], inp["q1_b2"][None, :]])
        ).astype(bf),
        "w1h": np.ascontiguousarray(
            inp["q1h_w1"].astype(np.float32)).astype(bf),
        "w2bh": np.ascontiguousarray(
            np.vstack([inp["q1h_w2"], inp["q1h_b2"][None, :]])
        ).astype(bf),
        "q2w1t": _q2_pretranspose(np.asarray(inp["q2_w1"], np.float32),
                                  16, 8, bf),
        "q2w2t": _q2_pretranspose(np.asarray(inp["q2_w2"], np.float32),
                                  8, 16, bf),
        "q2hw1t": _q2_pretranspose(np.asarray(inp["q2h_w1"], np.float32),
                                   16, 8, bf),
        "q2hw2t": _q2_pretranspose(np.asarray(inp["q2h_w2"], np.float32),
                                   8, 16, bf),
        "b1c": (inp["q2_b1"].astype(np.float32) * 256).reshape(1, CH)
            .astype(bf),
        "b2c": (inp["q2_b2"].astype(np.float32) * 256).reshape(1, CH2)
            .astype(bf),
        "w3c": np.ascontiguousarray(
            inp["q2_w3"].astype(np.float32).reshape(8, 128).T).astype(bf),
        "b3": inp["q2_b3"].astype(np.float32).reshape(1, 1),
        "b1ch": (inp["q2h_b1"].astype(np.float32) * 256).reshape(1, CH)
            .astype(bf),
        "b2ch": (inp["q2h_b2"].astype(np.float32) * 256).reshape(1, CH2)
            .astype(bf),
        "w3ch": np.ascontiguousarray(
            inp["q2h_w3"].astype(np.float32).reshape(8, 128).T).astype(bf),
        "b3h": inp["q2h_b3"].astype(np.float32).reshape(1, 1),
    }
    shared.update(_shared_consts())

    in_maps = []
    for c in range(NCORES):
        sid = set_ids[c * BC:(c + 1) * BC]          # [64, 64]
        iid = inst_ids[c * BC:(c + 1) * BC, 0]      # [64]
        ids_flat = np.concatenate(
            [sid.reshape(-1), iid,
             np.ones(R2 - R_REAL, np.int32)]).astype(np.int32)
        ids_ch = np.ascontiguousarray(ids_flat.reshape(NT, 128).T)
        mask = (sid != 0).astype(np.float32) * 256.0  # [64, 64]; x256 puts
        # the fp8-cast Q2 inputs in e4m3's normal range (e4m3 max is 240)
        gmask = np.zeros((128, 64), np.float32)  # cast to bf16 below
        for t in range(32):
            gmask[0:64, 2 * t] = mask[2 * t, :]
            gmask[64:128, 2 * t + 1] = mask[2 * t + 1, :]
        m = dict(shared)
        m["ids_ch"] = ids_ch
        m["gmask"] = gmask.astype(ml_dtypes.bfloat16)
        in_maps.append(m)
    return in_maps


def assemble_outputs(results):
    """results: list (per core) of dicts with 'out' [2, 128]."""
    setQ2 = np.zeros((B, 1), np.float32)
    setInst = np.zeros((B, 1), np.float32)
    ctxHat = np.zeros((B, 1), np.float32)
    ctxInstHat = np.zeros((B, 1), np.float32)
    for c in range(NCORES):
        o = np.asarray(results[c]["out"])
        setQ2[c * BC:(c + 1) * BC, 0] = o[0, 0:BC]
        setInst[c * BC:(c + 1) * BC, 0] = o[0, BC:2 * BC]
        ctxHat[c * BC:(c + 1) * BC, 0] = o[1, 0:BC]
        ctxInstHat[c * BC:(c + 1) * BC, 0] = o[1, BC:2 * BC]
    return (setQ2, setInst, ctxHat, ctxInstHat)


def run_cores(inputs, trace=False, **kw):
    nc = build_program()
    in_maps = make_in_maps(inputs)
    res = run_bass_kernel_spmd(nc, in_maps, list(range(NCORES)),
                               trace=trace, **kw)
    return assemble_outputs(res.results), res


def kernel(**inputs):
    outs, _ = run_cores(inputs, trace=False)
    return outs


# revision 57
# speedup vs baseline: 1.2215x; 1.0246x over previous
"""Trainium2 Bass kernel for nn_CNSYN_59528246723247.

Data-parallel over batch across 8 NeuronCores (64 batches/core), no
collectives. Per core, rows (4096 set + 64 inst, padded to 33x128) stream
through one interleaved pipeline:
  - ONE indirect-DMA gather per chunk ([128, 1100] bf16) from a host-joined
    bf16 table embj[i] = [emb[i] | emb[ca32[i,0]] | ... | emb[ca32[i,9]]]
    (data-independent O(V) join; bf16 halves DMA and doubles DVE rates)
  - context aggregation: scores/alpha on DVE in bf16 (2x perf mode),
    weighted sum + transpose to feature-major via bf16 PE accumulate-matmuls
    into one [100, 256] PSUM tile (ctx|set); drains split DVE/Act
  - Q1 MLPs feature-major on PE, all-bf16 (bias folded via ones-row); the
    masked sum over S becomes PE matmuls against a host-built 0/1 mask
  - Q2 MLPs feature-major on PE with HOST-PRETRANSPOSED contiguous bf16
    weight slabs (2-4KB DMA lines); both paths emission-interleaved
Outputs are assembled on host into the reference's 4-tuple.
"""

import sys

sys.path.insert(0, "/opt/trn_rl_repo")

from contextlib import ExitStack

import numpy as np
import ml_dtypes

import concourse.bass as bass
import concourse.mybir as mybir
import concourse.tile as tile
from concourse import bacc
from concourse.bass import IndirectOffsetOnAxis
from concourse.bass_utils import run_bass_kernel_spmd

# ---------------------------------------------------------------- dimensions
B, S, C, E = 512, 64, 10, 100
V, NH, CH = 100000, 1024, 2048
CH2 = CH // 2
NCORES = 8
BC = B // NCORES            # 64 batches per core
R_REAL = BC * S + BC        # 4160 rows per core: 4096 set + 64 inst
R2 = 4224                   # 33*128, padded row count
NT = R2 // 128              # 33 row chunks everywhere

f32 = mybir.dt.float32
bf16 = mybir.dt.bfloat16
fp8 = mybir.dt.float8e4
i32 = mybir.dt.int32
AF = mybir.ActivationFunctionType
ALU = mybir.AluOpType
AX = mybir.AxisListType

_CACHE = {}


# ---------------------------------------------------------------- program
def build_program():
    key = "nc"
    if key in _CACHE:
        return _CACHE[key]

    nc = bacc.Bacc("TRN2", debug=False, target_bir_lowering=False,
                   num_swdge_queues=1)

    def gather(out, in_, idx_ap):
        nc.gpsimd.indirect_dma_start(
            out=out, out_offset=None, in_=in_,
            in_offset=IndirectOffsetOnAxis(ap=idx_ap, axis=0),
        )

    # ---- DRAM parameters
    embj = nc.dram_tensor("embj", [V, E * (C + 1)], f32, kind="ExternalInput")
    ids_ch = nc.dram_tensor("ids_ch", [128, NT], i32, kind="ExternalInput")

    w1_d = nc.dram_tensor("w1", [E, E], bf16, kind="ExternalInput")
    w2b_d = nc.dram_tensor("w2b", [E + 1, NH], bf16, kind="ExternalInput")
    w1h_d = nc.dram_tensor("w1h", [E, E], bf16, kind="ExternalInput")
    w2bh_d = nc.dram_tensor("w2bh", [E + 1, NH], bf16, kind="ExternalInput")

    # Q2 weights, host-pretransposed into contiguous per-(m,k) 128x128 slabs
    q2w1_d = nc.dram_tensor("q2w1t", [128, 16 * NH], bf16,
                            kind="ExternalInput")
    q2w2_d = nc.dram_tensor("q2w2t", [128, 8 * CH], bf16,
                            kind="ExternalInput")
    q2hw1_d = nc.dram_tensor("q2hw1t", [128, 16 * NH], bf16,
                             kind="ExternalInput")
    q2hw2_d = nc.dram_tensor("q2hw2t", [128, 8 * CH], bf16,
                             kind="ExternalInput")

    b1c_d = nc.dram_tensor("b1c", [1, CH], bf16, kind="ExternalInput")
    b2c_d = nc.dram_tensor("b2c", [1, CH2], bf16, kind="ExternalInput")
    w3c_d = nc.dram_tensor("w3c", [128, 8], bf16, kind="ExternalInput")
    b3_d = nc.dram_tensor("b3", [1, 1], f32, kind="ExternalInput")
    b1ch_d = nc.dram_tensor("b1ch", [1, CH], bf16, kind="ExternalInput")
    b2ch_d = nc.dram_tensor("b2ch", [1, CH2], bf16, kind="ExternalInput")
    w3ch_d = nc.dram_tensor("w3ch", [128, 8], bf16, kind="ExternalInput")
    b3h_d = nc.dram_tensor("b3h", [1, 1], f32, kind="ExternalInput")

    gmask_d = nc.dram_tensor("gmask", [128, 64], bf16, kind="ExternalInput")
    e64_d = nc.dram_tensor("e64", [128, 64], bf16, kind="ExternalInput")
    i128_d = nc.dram_tensor("i128", [128, 128], bf16, kind="ExternalInput")
    i128f_d = nc.dram_tensor("i128f", [128, 128], f32, kind="ExternalInput")

    out_d = nc.dram_tensor("out", [2, 2 * BC], f32, kind="ExternalOutput")
    zgate_d = nc.dram_tensor("zgate", [1, NT], f32, kind="ExternalOutput")

    with tile.TileContext(nc) as tc, ExitStack() as ctx:
        const = ctx.enter_context(tc.tile_pool(name="const", bufs=1))
        gat = ctx.enter_context(tc.tile_pool(name="gat", bufs=4))
        work = ctx.enter_context(tc.tile_pool(name="work", bufs=2))
        big = ctx.enter_context(tc.tile_pool(name="big", bufs=1))
        y2p = ctx.enter_context(tc.tile_pool(name="y2p", bufs=2))
        q2wk = ctx.enter_context(tc.tile_pool(name="q2wk", bufs=1))

        # ---- load constants / small weights to SBUF (ids first; the first
        # gathers are emitted from the main loop right after the pools exist)
        ids_sb = const.tile([128, NT], i32)
        nc.sync.dma_start(ids_sb[:], ids_ch[:])
        w1_sb = const.tile([E, E], bf16)
        nc.sync.dma_start(w1_sb[:], w1_d[:])
        w2b_sb = const.tile([E + 1, NH], bf16)
        nc.sync.dma_start(w2b_sb[:], w2b_d[:])
        w1h_sb = const.tile([E, E], bf16)
        nc.sync.dma_start(w1h_sb[:], w1h_d[:])
        w2bh_sb = const.tile([E + 1, NH], bf16)
        nc.sync.dma_start(w2bh_sb[:], w2bh_d[:])
        gmask_sb = const.tile([128, 64], bf16)
        nc.sync.dma_start(gmask_sb[:], gmask_d[:])
        e64_sb = const.tile([128, 64], bf16)
        nc.sync.dma_start(e64_sb[:], e64_d[:])
        i128_sb = const.tile([128, 128], bf16)
        nc.sync.dma_start(i128_sb[:], i128_d[:])
        i128f_sb = const.tile([128, 128], f32)
        nc.sync.dma_start(i128f_sb[:], i128f_d[:])
        b1c_sb = const.tile([1, CH], bf16)
        nc.sync.dma_start(b1c_sb[:], b1c_d[:])
        b2c_sb = const.tile([1, CH2], bf16)
        nc.sync.dma_start(b2c_sb[:], b2c_d[:])
        w3c_sb = const.tile([128, 8], bf16)
        nc.sync.dma_start(w3c_sb[:], w3c_d[:])
        b3_sb = const.tile([1, 1], f32)
        nc.sync.dma_start(b3_sb[:], b3_d[:])
        b1ch_sb = const.tile([1, CH], bf16)
        nc.sync.dma_start(b1ch_sb[:], b1ch_d[:])
        b2ch_sb = const.tile([1, CH2], bf16)
        nc.sync.dma_start(b2ch_sb[:], b2ch_d[:])
        w3ch_sb = const.tile([128, 8], bf16)
        nc.sync.dma_start(w3ch_sb[:], w3ch_d[:])
        b3h_sb = const.tile([1, 1], f32)
        nc.sync.dma_start(b3h_sb[:], b3h_d[:])

        # Q2 layer-1 weights become SBUF-resident during the chunk phase;
        # layer-2 head slabs (first 4 m-blocks per path) too. The prefetch
        # DMAs are emitted inside the chunk loop, paced by a tiny gate DMA
        # that reads each chunk's gathered tile (keeps the shared DMA device
        # from starving the gathers).
        # all-bf16: fp8 anywhere on the value path costs ~3% (random-sign
        # dot products don't average quantization noise down)
        pdt = {"qs": bf16, "qc": bf16}
        w1t_res = {"qs": const.tile([128, 16 * NH], bf16, name="w1t_qs"),
                   "qc": const.tile([128, 16 * NH], bf16, name="w1t_qc")}
        w2t_res = {"qs": const.tile([128, 8 * CH], bf16, name="w2t_qs"),
                   "qc": const.tile([128, 8 * CH], bf16, name="w2t_qc")}
        w1t_dram = {"qs": q2w1_d, "qc": q2hw1_d}
        w2t_dram = {"qs": q2w2_d, "qc": q2hw2_d}
        slab_jobs = []
        for m in range(16):
            for pn in ("qs", "qc"):
                slab_jobs.append((w1t_res[pn][:, NH * m:NH * (m + 1)],
                                  w1t_dram[pn][:, NH * m:NH * (m + 1)]))
        for m in range(8):
            for pn in ("qs", "qc"):
                slab_jobs.append((w2t_res[pn][:, CH * m:CH * (m + 1)],
                                  w2t_dram[pn][:, CH * m:CH * (m + 1)]))
        ones1 = {}
        for pn, dt_ in (("qs", bf16), ("qc", bf16)):
            o = const.tile([1, 128], dt_, name="ones_" + pn)
            nc.vector.memset(o[:], 1.0)
            ones1[pn] = o

        # xT activations, feature-major, interleaved per chunk as
        # [ctx 128 | set 128] so one Act instr drains both PSUM transposes.
        # Ring buffer: L1 consumes within ~6 chunks, keep an 8-chunk window.
        xt_all = big.tile([E, 8 * 256], bf16)
        xt_v = xt_all[:].rearrange("e (t x c) -> e t x c", t=8, x=2)

        # single shared PSUM pool; tags budget all 8 banks:
        #   qsseg 1 + qcseg 1 + xtps 2 + l2 (2x 2-bank) 4 = 8
        ps = ctx.enter_context(tc.tile_pool(name="ps", bufs=1, space="PSUM"))

        # h1 rings: L2 lags L1 by <8 chunks, keep two 512-col L1 blocks
        h1s = big.tile([E + 1, 1024], bf16)
        h1c = big.tile([E + 1, 1024], bf16)
        # engine ops need 32-aligned start partition: set rows 96..100 to
        # 1.0; the L1 relu overwrites rows 96..99 afterwards.
        nc.gpsimd.memset(h1s[96:E + 1, :], 1.0)
        nc.gpsimd.memset(h1c[96:E + 1, :], 1.0)
        paths = (
            ("qs", 1, h1s, w1_sb, w2b_sb),
            ("qc", 0, h1c, w1h_sb, w2bh_sb),
        )
        segaccs, instaccs = {}, {}
        for name, _, _, _, _ in paths:
            segaccs[name] = ps.tile([128, 512], f32, name=name + "_seg",
                                    tag=name + "seg", bufs=1)

        ec_tiles, scaled_tiles = {}, {}

        def emit_gather(t):
            ec = gat.tile([128, E * (C + 1)], f32, name="ec", bufs=4)
            gather(ec[:], embj[:, :], ids_sb[:, t:t + 1])
            ec_tiles[t] = ec

        def emit_scores(t):
            ec = ec_tiles[t]
            ent = ec[:, 0:E]
            ctx_ap = ec[:, E:E * (C + 1)]

            # scores s[p,c] = <ctx[p,c,:], ent[p,:]>; z is cancellation-
            # sensitive, so the whole score path stays fp32 (split DVE/Pool)
            PD = 6
            prod = work.tile([128, C * E], f32)
            nc.vector.tensor_tensor(
                prod[:, 0:PD * E].rearrange("p (c d) -> p c d", c=PD),
                ctx_ap[:, 0:PD * E].rearrange("p (c d) -> p c d", c=PD),
                ent.unsqueeze(1).to_broadcast([128, PD, E]),
                op=ALU.mult,
            )
            nc.gpsimd.tensor_tensor(
                prod[:, PD * E:].rearrange("p (c d) -> p c d", c=C - PD),
                ctx_ap[:, PD * E:].rearrange("p (c d) -> p c d", c=C - PD),
                ent.unsqueeze(1).to_broadcast([128, C - PD, E]),
                op=ALU.mult,
            )
            s_all = work.tile([128, C], f32)
            nc.vector.tensor_reduce(
                s_all[:], prod[:].rearrange("p (c d) -> p c d", c=C),
                axis=AX.X, op=ALU.add,
            )
            z = work.tile([128, 1], f32)
            nc.vector.tensor_reduce(z[:], s_all[:], axis=AX.X, op=ALU.add)
            rz = work.tile([128, 1], f32)
            nc.vector.reciprocal(rz[:], z[:])
            alpha = work.tile([128, C], f32)
            nc.vector.tensor_scalar(alpha[:], s_all[:], rz[:], None,
                                    op0=ALU.mult)
            # scaled[p, c, :] = alpha[p, c] * ctx[p, c, :]  (bf16 out: only
            # relative error, safe after normalization). 2 blocks on DVE,
            # 8 on Pool to balance engine load.
            CD = 6
            scaled = work.tile([128, C * E], bf16)
            nc.vector.tensor_tensor(
                scaled[:, 0:CD * E].rearrange("p (c d) -> p c d", c=CD),
                ctx_ap[:, 0:CD * E].rearrange("p (c d) -> p c d", c=CD),
                alpha[:, 0:CD].unsqueeze(2).to_broadcast([128, CD, E]),
                op=ALU.mult,
            )
            nc.gpsimd.tensor_tensor(
                scaled[:, CD * E:].rearrange("p (c d) -> p c d", c=C - CD),
                ctx_ap[:, CD * E:].rearrange("p (c d) -> p c d", c=C - CD),
                alpha[:, CD:].unsqueeze(2).to_broadcast([128, C - CD, E]),
                op=ALU.mult,
            )
            scaled_tiles[t] = scaled

        def emit_transposes(t):
            ec = ec_tiles.pop(t)
            ent = ec[:, 0:E]
            scaled = scaled_tiles.pop(t)
            # one PSUM tile holds [ctx-agg^T | ent^T] for this chunk
            xt_ps = ps.tile([E, 256], f32, tag="xtps", bufs=2)
            for c in range(C):
                nc.tensor.matmul(
                    xt_ps[:, 0:128], lhsT=scaled[:, c * E:(c + 1) * E],
                    rhs=i128_sb[:],
                    start=(c == 0), stop=(c == C - 1),
                )
            nc.tensor.matmul(xt_ps[:, 128:256], lhsT=ent, rhs=i128f_sb[:],
                             start=True, stop=True)
            nc.scalar.copy(
                xt_all[:, 256 * (t % 8):256 * (t % 8 + 1)], xt_ps[:])

        def emit_L1(j):
            jj = j * 512
            w = min(512, R2 - jj)
            nch = w // 128
            s0 = (4 * j) % 8
            hj = 512 * (j % 2)
            for name, xsel, h1, w1s, _ in paths:
                psl = ps.tile([E, 512], f32, name=name + "_l1",
                              tag="l2", bufs=2)
                nc.tensor.matmul(psl[:, :w], lhsT=w1s[:],
                                 rhs=xt_v[:, s0:s0 + nch, xsel, :],
                                 start=True, stop=True)
                nc.scalar.activation(h1[0:E, hj:hj + w], psl[:, :w], AF.Relu)

        def emit_L2(t):
            for name, _, h1, _, w2bs in paths:
                segacc = segaccs[name]
                yab = ps.tile([128, NH], f32, name=name + "_l2",
                              tag="l2", bufs=2)
                lhsT = h1[:, 128 * (t % 8):128 * (t % 8 + 1)]
                nc.tensor.matmul(yab[:, 0:512], lhsT=lhsT,
                                 rhs=w2bs[:, 0:512], start=True, stop=True)
                nc.tensor.matmul(yab[:, 512:NH], lhsT=lhsT,
                                 rhs=w2bs[:, 512:NH], start=True, stop=True)
                y2 = y2p.tile([128, NH], bf16, name=name + "_y2", tag="y2",
                              bufs=2)
                if t >= NT - 6 and name == "qc":
                    nc.vector.tensor_scalar(y2[:], yab[:], 0.0, None,
                                            op0=ALU.max)
                else:
                    nc.scalar.activation(y2[:], yab[:], AF.Relu)
                if t < 32:
                    for f in range(8):
                        nc.tensor.matmul(
                            segacc[:, 64 * f + 2 * t:64 * f + 2 * t + 2],
                            lhsT=y2[:, 128 * f:128 * (f + 1)],
                            rhs=gmask_sb[:, 2 * t:2 * t + 2],
                            start=True, stop=True,
                        )
                else:
                    instacc = ps.tile([128, 512], f32, name=name + "_ins",
                                      tag="l2", bufs=2)
                    instaccs[name] = instacc
                    for f in range(8):
                        nc.tensor.matmul(
                            instacc[:, 64 * f:64 * (f + 1)],
                            lhsT=y2[:, 128 * f:128 * (f + 1)],
                            rhs=e64_sb[:],
                            start=True, stop=True,
                        )

        emit_gather(0)
        emit_gather(1)
        emit_gather(2)
        n_slab = 0
        for t in range(NT):
            if t % 4 == 1 and t >= 5:
                emit_L1((t - 5) // 4)
            if t >= 5:
                emit_L2(t - 5)
            if t + 3 < NT:
                emit_gather(t + 3)
            emit_scores(t)
            # paced Q2-weight prefetch: gate on this chunk's gathered tile,
            # then ship slabs on the idle SP queue
            if 1 <= t <= 32:
                nc.sync.dma_start(zgate_d[:, t:t + 1],
                                  ec_tiles[min(t + 2, NT - 1)][0:1, 0:1])
                target = min(len(slab_jobs), (8 * t) // 5)
                while n_slab < target:
                    dst, srcap = slab_jobs[n_slab]
                    nc.sync.dma_start(dst, srcap)
                    n_slab += 1
            if t >= 1:
                emit_transposes(t - 1)
            if t == 32:
                emit_L1(7)
        emit_transposes(NT - 1)
        while n_slab < len(slab_jobs):
            dst, srcap = slab_jobs[n_slab]
            nc.sync.dma_start(dst, srcap)
            n_slab += 1
        emit_L1(8)
        for t in range(NT - 5, NT):
            emit_L2(t)

        def build_x2(name):
            segacc, instacc = segaccs[name], instaccs[name]
            # q2 inputs: [embed | embed + inst-embed], feature-major blocks
            # hw allows only one PSUM operand per instruction: stage the
            # inst transpose in SBUF first
            iT = q2wk.tile([128, 512], bf16, name=name + "_iT")
            nc.scalar.copy(iT[:], instacc[:])
            x2 = q2wk.tile([128, NH], pdt[name], name=name + "_x2")
            x2v = x2[:].rearrange("p (f c) -> p f c", f=8)
            sv = segacc[:].rearrange("p (f c) -> p f c", f=8)
            iv = iT[:].rearrange("p (f c) -> p f c", f=8)
            nc.scalar.copy(x2v[:, :, 0:64], sv[:])
            nc.vector.tensor_tensor(x2v[:, :, 64:128], sv[:], iv[:],
                                    op=ALU.add)
            return x2

        x2_set = build_x2("qs")
        x2_ctx = build_x2("qc")

        # ---------------- phase D: Q2 MLPs
        def q2_mlp(x2, name, b1row, b2row, w3s, b3s, out_row):
            # generator-based so the two paths can be emitted interleaved;
            # 4 m-blocks share one [128,512] PSUM tile so PE runs ~1.7us
            # bursts per drain; biases enter as a K=1 matmul of ones
            w1t_sb = w1t_res[name]
            w2t_sb = w2t_res[name]
            hq = q2wk.tile([128, CH], pdt[name], name=name + "_hq")
            for mg in range(4):
                psq = ps.tile([128, 512], f32, name=name + "_p1",
                              tag="l2", bufs=2)
                for mi in range(4):
                    m = 4 * mg + mi
                    for k in range(8):
                        nc.tensor.matmul(
                            psq[:, 128 * mi:128 * (mi + 1)],
                            lhsT=w1t_sb[:, NH * m + 128 * k:
                                        NH * m + 128 * (k + 1)],
                            rhs=x2[:, 128 * k:128 * (k + 1)],
                            start=(k == 0), stop=False,
                        )
                    nc.tensor.matmul(
                        psq[:, 128 * mi:128 * (mi + 1)],
                        lhsT=b1row[:, 128 * m:128 * (m + 1)],
                        rhs=ones1[name][:], start=False, stop=True,
                    )
                hsc = 1.0
                if mg % 2 == 0:
                    nc.scalar.activation(hq[:, 512 * mg:512 * (mg + 1)],
                                         psq[:], AF.Relu, scale=hsc)
                else:
                    nc.vector.tensor_scalar(hq[:, 512 * mg:512 * (mg + 1)],
                                            psq[:], hsc, 0.0,
                                            op0=ALU.mult, op1=ALU.max)
                yield
            h2 = q2wk.tile([128, CH2], bf16, name=name + "_h2")
            for mg in range(2):
                psq = ps.tile([128, 512], f32, name=name + "_p2",
                              tag="l2", bufs=2)
                for mi in range(4):
                    m = 4 * mg + mi
                    for k in range(16):
                        nc.tensor.matmul(
                            psq[:, 128 * mi:128 * (mi + 1)],
                            lhsT=w2t_sb[:, CH * m + 128 * k:
                                        CH * m + 128 * (k + 1)],
                            rhs=hq[:, 128 * k:128 * (k + 1)],
                            start=(k == 0), stop=False,
                        )
                    nc.tensor.matmul(
                        psq[:, 128 * mi:128 * (mi + 1)],
                        lhsT=b2row[:, 128 * m:128 * (m + 1)],
                        rhs=ones1[name][:], start=False, stop=True,
                    )
                # undo the activation scaling (qs: 2048x, qc: 512x)
                usc = 1.0 / 256
                if mg % 2 == 0:
                    nc.vector.tensor_scalar(h2[:, 512 * mg:512 * (mg + 1)],
                                            psq[:], usc, 0.0,
                                            op0=ALU.mult, op1=ALU.max)
                else:
                    nc.scalar.activation(h2[:, 512 * mg:512 * (mg + 1)],
                                         psq[:], AF.Relu, scale=usc)
                yield
            ps3 = ps.tile([1, 128], f32, name=name + "_p3",
                          tag="xtps", bufs=2)
            for k in range(8):
                nc.tensor.matmul(
                    ps3[:],
                    lhsT=w3s[:, k:k + 1],
                    rhs=h2[:, 128 * k:128 * (k + 1)],
                    start=(k == 0), stop=(k == 7),
                )
            osb = q2wk.tile([1, 128], f32, name=name + "_o")
            nc.scalar.activation(osb[:], ps3[:], AF.Identity, bias=b3s[:])
            nc.sync.dma_start(out_row, osb[:])
            yield

        gens = [
            q2_mlp(x2_set, "qs", b1c_sb, b2c_sb, w3c_sb, b3_sb,
                   out_d[0:1, :]),
            q2_mlp(x2_ctx, "qc", b1ch_sb, b2ch_sb, w3ch_sb, b3h_sb,
                   out_d[1:2, :]),
        ]
        alive = list(gens)
        while alive:
            for g in list(alive):
                try:
                    next(g)
                    next(g)
                except StopIteration:
                    alive.remove(g)

    nc.compile()
    _CACHE[key] = nc
    return nc


# ---------------------------------------------------------------- host prep
def _q2_pretranspose(w, n_m, n_k, dt):
    # [p, m*(n_k*128) + k*128 + c] = w[128k+p, 128m+c]
    kdim, mdim = w.shape
    assert kdim == n_k * 128 and mdim == n_m * 128
    wt = w.reshape(n_k, 128, n_m, 128).transpose(1, 2, 0, 3)
    return np.ascontiguousarray(wt.reshape(128, n_m * n_k * 128)).astype(dt)


def _shared_consts():
    if "consts" in _CACHE:
        return _CACHE["consts"]
    c = {
        "e64": (np.eye(128, 64) * 256.0).astype(ml_dtypes.bfloat16),
        "i128": np.eye(128).astype(ml_dtypes.bfloat16),
        "i128f": np.eye(128, dtype=np.float32),
    }
    _CACHE["consts"] = c
    return c


def make_in_maps(inputs):
    """inputs: dict of FULL numpy arrays keyed as in setup_inputs()."""
    inp = {k: np.asarray(v) for k, v in inputs.items()}
    set_ids = inp["set_ids"].astype(np.int32)
    inst_ids = inp["inst_ids"].astype(np.int32)
    ca32 = np.ascontiguousarray(inp["contex_array"].astype(np.int32))
    emb = np.ascontiguousarray(inp["emb"].astype(np.float32))

    embj = np.empty((V, (C + 1) * E), np.float32)
    embj[:, :E] = emb
    for c in range(C):
        embj[:, (c + 1) * E:(c + 2) * E] = emb[ca32[:, c], :]
    bf = ml_dtypes.bfloat16
    f8 = ml_dtypes.float8_e4m3
    shared = {
        "embj": embj,
        "w1": np.ascontiguousarray(inp["q1_w1"].astype(np.float32)).astype(bf),
        "w2b": np.ascontiguousarray(
            np.vstack([inp["q1_w2"], inp["q1_b2"][None, :]])
        ).astype(bf),
        "w1h": np.ascontiguousarray(
            inp["q1h_w1"].astype(np.float32)).astype(bf),
        "w2bh": np.ascontiguousarray(
            np.vstack([inp["q1h_w2"], inp["q1h_b2"][None, :]])
        ).astype(bf),
        "q2w1t": _q2_pretranspose(np.asarray(inp["q2_w1"], np.float32),
                                  16, 8, bf),
        "q2w2t": _q2_pretranspose(np.asarray(inp["q2_w2"], np.float32),
                                  8, 16, bf),
        "q2hw1t": _q2_pretranspose(np.asarray(inp["q2h_w1"], np.float32),
                                   16, 8, bf),
        "q2hw2t": _q2_pretranspose(np.asarray(inp["q2h_w2"], np.float32),
                                   8, 16, bf),
        "b1c": (inp["q2_b1"].astype(np.float32) * 256).reshape(1, CH)
            .astype(bf),
        "b2c": (inp["q2_b2"].astype(np.float32) * 256).reshape(1, CH2)
            .astype(bf),
        "w3c": np.ascontiguousarray(
            inp["q2_w3"].astype(np.float32).reshape(8, 128).T).astype(bf),
        "b3": inp["q2_b3"].astype(np.float32).reshape(1, 1),
        "b1ch": (inp["q2h_b1"].astype(np.float32) * 256).reshape(1, CH)
            .astype(bf),
        "b2ch": (inp["q2h_b2"].astype(np.float32) * 256).reshape(1, CH2)
            .astype(bf),
        "w3ch": np.ascontiguousarray(
            inp["q2h_w3"].astype(np.float32).reshape(8, 128).T).astype(bf),
        "b3h": inp["q2h_b3"].astype(np.float32).reshape(1, 1),
    }
    shared.update(_shared_consts())

    in_maps = []
    for c in range(NCORES):
        sid = set_ids[c * BC:(c + 1) * BC]          # [64, 64]
        iid = inst_ids[c * BC:(c + 1) * BC, 0]      # [64]
        ids_flat = np.concatenate(
            [sid.reshape(-1), iid,
             np.ones(R2 - R_REAL, np.int32)]).astype(np.int32)
        ids_ch = np.ascontiguousarray(ids_flat.reshape(NT, 128).T)
        mask = (sid != 0).astype(np.float32) * 256.0  # [64, 64]; x256 puts
        # the fp8-cast Q2 inputs in e4m3's normal range (e4m3 max is 240)
        gmask = np.zeros((128, 64), np.float32)  # cast to bf16 below
        for t in range(32):
            gmask[0:64, 2 * t] = mask[2 * t, :]
            gmask[64:128, 2 * t + 1] = mask[2 * t + 1, :]
        m = dict(shared)
        m["ids_ch"] = ids_ch
        m["gmask"] = gmask.astype(ml_dtypes.bfloat16)
        in_maps.append(m)
    return in_maps


def assemble_outputs(results):
    """results: list (per core) of dicts with 'out' [2, 128]."""
    setQ2 = np.zeros((B, 1), np.float32)
    setInst = np.zeros((B, 1), np.float32)
    ctxHat = np.zeros((B, 1), np.float32)
    ctxInstHat = np.zeros((B, 1), np.float32)
    for c in range(NCORES):
        o = np.asarray(results[c]["out"])
        setQ2[c * BC:(c + 1) * BC, 0] = o[0, 0:BC]
        setInst[c * BC:(c + 1) * BC, 0] = o[0, BC:2 * BC]
        ctxHat[c * BC:(c + 1) * BC, 0] = o[1, 0:BC]
        ctxInstHat[c * BC:(c + 1) * BC, 0] = o[1, BC:2 * BC]
    return (setQ2, setInst, ctxHat, ctxInstHat)


def run_cores(inputs, trace=False, **kw):
    nc = build_program()
    in_maps = make_in_maps(inputs)
    res = run_bass_kernel_spmd(nc, in_maps, list(range(NCORES)),
                               trace=trace, **kw)
    return assemble_outputs(res.results), res


def kernel(**inputs):
    outs, _ = run_cores(inputs, trace=False)
    return outs


# revision 66
# speedup vs baseline: 1.2317x; 1.0084x over previous
"""Trainium2 Bass kernel for nn_CNSYN_59528246723247.

Data-parallel over batch across 8 NeuronCores (64 batches/core), no
collectives. Per core, rows (4096 set + 64 inst, padded to 33x128) stream
through one interleaved pipeline:
  - ONE indirect-DMA gather per chunk ([128, 1100] bf16) from a host-joined
    bf16 table embj[i] = [emb[i] | emb[ca32[i,0]] | ... | emb[ca32[i,9]]]
    (data-independent O(V) join; bf16 halves DMA and doubles DVE rates)
  - context aggregation: scores/alpha on DVE in bf16 (2x perf mode),
    weighted sum + transpose to feature-major via bf16 PE accumulate-matmuls
    into one [100, 256] PSUM tile (ctx|set); drains split DVE/Act
  - Q1 MLPs feature-major on PE, all-bf16 (bias folded via ones-row); the
    masked sum over S becomes PE matmuls against a host-built 0/1 mask
  - Q2 MLPs feature-major on PE with HOST-PRETRANSPOSED contiguous bf16
    weight slabs (2-4KB DMA lines); both paths emission-interleaved
Outputs are assembled on host into the reference's 4-tuple.
"""

import sys

sys.path.insert(0, "/opt/trn_rl_repo")

from contextlib import ExitStack

import numpy as np
import ml_dtypes

import concourse.bass as bass
import concourse.mybir as mybir
import concourse.tile as tile
from concourse import bacc
from concourse.bass import IndirectOffsetOnAxis
from concourse.bass_utils import run_bass_kernel_spmd

# ---------------------------------------------------------------- dimensions
B, S, C, E = 512, 64, 10, 100
V, NH, CH = 100000, 1024, 2048
CH2 = CH // 2
NCORES = 8
BC = B // NCORES            # 64 batches per core
R_REAL = BC * S + BC        # 4160 rows per core: 4096 set + 64 inst
R2 = 4224                   # 33*128, padded row count
NT = R2 // 128              # 33 row chunks everywhere

f32 = mybir.dt.float32
bf16 = mybir.dt.bfloat16
fp8 = mybir.dt.float8e4
i32 = mybir.dt.int32
AF = mybir.ActivationFunctionType
ALU = mybir.AluOpType
AX = mybir.AxisListType

_CACHE = {}


# ---------------------------------------------------------------- program
def build_program():
    key = "nc"
    if key in _CACHE:
        return _CACHE[key]

    nc = bacc.Bacc("TRN2", debug=False, target_bir_lowering=False,
                   num_swdge_queues=1)

    def gather(out, in_, idx_ap):
        nc.gpsimd.indirect_dma_start(
            out=out, out_offset=None, in_=in_,
            in_offset=IndirectOffsetOnAxis(ap=idx_ap, axis=0),
        )

    # ---- DRAM parameters
    embj = nc.dram_tensor("embj", [V, E * (C + 1)], f32, kind="ExternalInput")
    ids_ch = nc.dram_tensor("ids_ch", [128, NT], i32, kind="ExternalInput")

    w1_d = nc.dram_tensor("w1", [E, E], bf16, kind="ExternalInput")
    w2b_d = nc.dram_tensor("w2b", [E + 1, NH], bf16, kind="ExternalInput")
    w1h_d = nc.dram_tensor("w1h", [E, E], bf16, kind="ExternalInput")
    w2bh_d = nc.dram_tensor("w2bh", [E + 1, NH], bf16, kind="ExternalInput")

    # Q2 weights, host-pretransposed into contiguous per-(m,k) 128x128 slabs
    q2w1_d = nc.dram_tensor("q2w1t", [128, 16 * NH], bf16,
                            kind="ExternalInput")
    q2w2_d = nc.dram_tensor("q2w2t", [128, 8 * CH], bf16,
                            kind="ExternalInput")
    q2hw1_d = nc.dram_tensor("q2hw1t", [128, 16 * NH], bf16,
                             kind="ExternalInput")
    q2hw2_d = nc.dram_tensor("q2hw2t", [128, 8 * CH], bf16,
                             kind="ExternalInput")

    b1c_d = nc.dram_tensor("b1c", [1, CH], bf16, kind="ExternalInput")
    b2c_d = nc.dram_tensor("b2c", [1, CH2], bf16, kind="ExternalInput")
    w3c_d = nc.dram_tensor("w3c", [128, 8], bf16, kind="ExternalInput")
    b3_d = nc.dram_tensor("b3", [1, 1], f32, kind="ExternalInput")
    b1ch_d = nc.dram_tensor("b1ch", [1, CH], bf16, kind="ExternalInput")
    b2ch_d = nc.dram_tensor("b2ch", [1, CH2], bf16, kind="ExternalInput")
    w3ch_d = nc.dram_tensor("w3ch", [128, 8], bf16, kind="ExternalInput")
    b3h_d = nc.dram_tensor("b3h", [1, 1], f32, kind="ExternalInput")

    gmask_d = nc.dram_tensor("gmask", [128, 64], bf16, kind="ExternalInput")
    e64_d = nc.dram_tensor("e64", [128, 64], bf16, kind="ExternalInput")
    i128_d = nc.dram_tensor("i128", [128, 128], bf16, kind="ExternalInput")
    i128f_d = nc.dram_tensor("i128f", [128, 128], f32, kind="ExternalInput")

    out_d = nc.dram_tensor("out", [2, 2 * BC], f32, kind="ExternalOutput")
    zgate_d = nc.dram_tensor("zgate", [1, NT], f32, kind="ExternalOutput")

    with tile.TileContext(nc) as tc, ExitStack() as ctx:
        const = ctx.enter_context(tc.tile_pool(name="const", bufs=1))
        gat = ctx.enter_context(tc.tile_pool(name="gat", bufs=4))
        work = ctx.enter_context(tc.tile_pool(name="work", bufs=2))
        big = ctx.enter_context(tc.tile_pool(name="big", bufs=1))
        y2p = ctx.enter_context(tc.tile_pool(name="y2p", bufs=2))
        q2wk = ctx.enter_context(tc.tile_pool(name="q2wk", bufs=1))

        # ---- load constants / small weights to SBUF (ids first; the first
        # gathers are emitted from the main loop right after the pools exist)
        ids_sb = const.tile([128, NT], i32)
        nc.sync.dma_start(ids_sb[:], ids_ch[:])
        w1_sb = const.tile([E, E], bf16)
        nc.sync.dma_start(w1_sb[:], w1_d[:])
        w2b_sb = const.tile([E + 1, NH], bf16)
        nc.sync.dma_start(w2b_sb[:], w2b_d[:])
        w1h_sb = const.tile([E, E], bf16)
        nc.sync.dma_start(w1h_sb[:], w1h_d[:])
        w2bh_sb = const.tile([E + 1, NH], bf16)
        nc.sync.dma_start(w2bh_sb[:], w2bh_d[:])
        gmask_sb = const.tile([128, 64], bf16)
        nc.sync.dma_start(gmask_sb[:], gmask_d[:])
        e64_sb = const.tile([128, 64], bf16)
        nc.sync.dma_start(e64_sb[:], e64_d[:])
        i128_sb = const.tile([128, 128], bf16)
        nc.sync.dma_start(i128_sb[:], i128_d[:])
        i128f_sb = const.tile([128, 128], f32)
        nc.sync.dma_start(i128f_sb[:], i128f_d[:])
        b1c_sb = const.tile([1, CH], bf16)
        nc.sync.dma_start(b1c_sb[:], b1c_d[:])
        b2c_sb = const.tile([1, CH2], bf16)
        nc.sync.dma_start(b2c_sb[:], b2c_d[:])
        w3c_sb = const.tile([128, 8], bf16)
        nc.sync.dma_start(w3c_sb[:], w3c_d[:])
        b3_sb = const.tile([1, 1], f32)
        nc.sync.dma_start(b3_sb[:], b3_d[:])
        b1ch_sb = const.tile([1, CH], bf16)
        nc.sync.dma_start(b1ch_sb[:], b1ch_d[:])
        b2ch_sb = const.tile([1, CH2], bf16)
        nc.sync.dma_start(b2ch_sb[:], b2ch_d[:])
        w3ch_sb = const.tile([128, 8], bf16)
        nc.sync.dma_start(w3ch_sb[:], w3ch_d[:])
        b3h_sb = const.tile([1, 1], f32)
        nc.sync.dma_start(b3h_sb[:], b3h_d[:])

        # Q2 layer-1 weights become SBUF-resident during the chunk phase;
        # layer-2 head slabs (first 4 m-blocks per path) too. The prefetch
        # DMAs are emitted inside the chunk loop, paced by a tiny gate DMA
        # that reads each chunk's gathered tile (keeps the shared DMA device
        # from starving the gathers).
        # all-bf16: fp8 anywhere on the value path costs ~3% (random-sign
        # dot products don't average quantization noise down)
        pdt = {"qs": bf16, "qc": bf16}
        w1t_res = {"qs": const.tile([128, 16 * NH], bf16, name="w1t_qs"),
                   "qc": const.tile([128, 16 * NH], bf16, name="w1t_qc")}
        w2t_res = {"qs": const.tile([128, 8 * CH], bf16, name="w2t_qs"),
                   "qc": const.tile([128, 8 * CH], bf16, name="w2t_qc")}
        w1t_dram = {"qs": q2w1_d, "qc": q2hw1_d}
        w2t_dram = {"qs": q2w2_d, "qc": q2hw2_d}
        slab_jobs = []
        for m in range(16):
            for pn in ("qs", "qc"):
                slab_jobs.append((w1t_res[pn][:, NH * m:NH * (m + 1)],
                                  w1t_dram[pn][:, NH * m:NH * (m + 1)]))
        for m in range(8):
            for pn in ("qs", "qc"):
                slab_jobs.append((w2t_res[pn][:, CH * m:CH * (m + 1)],
                                  w2t_dram[pn][:, CH * m:CH * (m + 1)]))
        ones1 = {}
        for pn, dt_ in (("qs", bf16), ("qc", bf16)):
            o = const.tile([1, 128], dt_, name="ones_" + pn)
            nc.vector.memset(o[:], 1.0)
            ones1[pn] = o

        # xT activations, feature-major, interleaved per chunk as
        # [ctx 128 | set 128] so one Act instr drains both PSUM transposes.
        # Ring buffer: L1 consumes within ~6 chunks, keep an 8-chunk window.
        xt_all = big.tile([E, 8 * 256], bf16)
        xt_v = xt_all[:].rearrange("e (t x c) -> e t x c", t=8, x=2)

        # single shared PSUM pool; tags budget all 8 banks:
        #   qsseg 1 + qcseg 1 + xtps 2 + l2 (2x 2-bank) 4 = 8
        ps = ctx.enter_context(tc.tile_pool(name="ps", bufs=1, space="PSUM"))

        # h1 rings: L2 lags L1 by <8 chunks, keep two 512-col L1 blocks
        h1s = big.tile([E + 1, 1024], bf16)
        h1c = big.tile([E + 1, 1024], bf16)
        # engine ops need 32-aligned start partition: set rows 96..100 to
        # 1.0; the L1 relu overwrites rows 96..99 afterwards.
        nc.gpsimd.memset(h1s[96:E + 1, :], 1.0)
        nc.gpsimd.memset(h1c[96:E + 1, :], 1.0)
        paths = (
            ("qs", 1, h1s, w1_sb, w2b_sb),
            ("qc", 0, h1c, w1h_sb, w2bh_sb),
        )
        segaccs, instaccs = {}, {}
        for name, _, _, _, _ in paths:
            segaccs[name] = ps.tile([128, 512], f32, name=name + "_seg",
                                    tag=name + "seg", bufs=1)

        ec_tiles, scaled_tiles = {}, {}

        def emit_gather(t):
            ec = gat.tile([128, E * (C + 1)], f32, name="ec", bufs=4)
            gather(ec[:], embj[:, :], ids_sb[:, t:t + 1])
            ec_tiles[t] = ec

        def emit_scores(t):
            ec = ec_tiles[t]
            ent = ec[:, 0:E]
            ctx_ap = ec[:, E:E * (C + 1)]

            # scores s[p,c] = <ctx[p,c,:], ent[p,:]>; z is cancellation-
            # sensitive, so the whole score path stays fp32 (split DVE/Pool)
            PD = 6
            prod = work.tile([128, C * E], f32)
            nc.vector.tensor_tensor(
                prod[:, 0:PD * E].rearrange("p (c d) -> p c d", c=PD),
                ctx_ap[:, 0:PD * E].rearrange("p (c d) -> p c d", c=PD),
                ent.unsqueeze(1).to_broadcast([128, PD, E]),
                op=ALU.mult,
            )
            nc.gpsimd.tensor_tensor(
                prod[:, PD * E:].rearrange("p (c d) -> p c d", c=C - PD),
                ctx_ap[:, PD * E:].rearrange("p (c d) -> p c d", c=C - PD),
                ent.unsqueeze(1).to_broadcast([128, C - PD, E]),
                op=ALU.mult,
            )
            s_all = work.tile([128, C], f32)
            nc.vector.tensor_reduce(
                s_all[:], prod[:].rearrange("p (c d) -> p c d", c=C),
                axis=AX.X, op=ALU.add,
            )
            z = work.tile([128, 1], f32)
            nc.vector.tensor_reduce(z[:], s_all[:], axis=AX.X, op=ALU.add)
            rz = work.tile([128, 1], f32)
            nc.vector.reciprocal(rz[:], z[:])
            alpha = work.tile([128, C], f32)
            nc.vector.tensor_scalar(alpha[:], s_all[:], rz[:], None,
                                    op0=ALU.mult)
            # scaled[p, c, :] = alpha[p, c] * ctx[p, c, :]  (bf16 out: only
            # relative error, safe after normalization). 2 blocks on DVE,
            # 8 on Pool to balance engine load.
            CD = 6
            scaled = work.tile([128, C * E], bf16)
            nc.vector.tensor_tensor(
                scaled[:, 0:CD * E].rearrange("p (c d) -> p c d", c=CD),
                ctx_ap[:, 0:CD * E].rearrange("p (c d) -> p c d", c=CD),
                alpha[:, 0:CD].unsqueeze(2).to_broadcast([128, CD, E]),
                op=ALU.mult,
            )
            nc.gpsimd.tensor_tensor(
                scaled[:, CD * E:].rearrange("p (c d) -> p c d", c=C - CD),
                ctx_ap[:, CD * E:].rearrange("p (c d) -> p c d", c=C - CD),
                alpha[:, CD:].unsqueeze(2).to_broadcast([128, C - CD, E]),
                op=ALU.mult,
            )
            scaled_tiles[t] = scaled

        def emit_transposes(t):
            ec = ec_tiles.pop(t)
            ent = ec[:, 0:E]
            scaled = scaled_tiles.pop(t)
            # one PSUM tile holds [ctx-agg^T | ent^T] for this chunk
            xt_ps = ps.tile([E, 256], f32, tag="xtps", bufs=2)
            for c in range(C):
                nc.tensor.matmul(
                    xt_ps[:, 0:128], lhsT=scaled[:, c * E:(c + 1) * E],
                    rhs=i128_sb[:],
                    start=(c == 0), stop=(c == C - 1),
                )
            nc.tensor.matmul(xt_ps[:, 128:256], lhsT=ent, rhs=i128f_sb[:],
                             start=True, stop=True)
            nc.scalar.copy(
                xt_all[:, 256 * (t % 8):256 * (t % 8 + 1)], xt_ps[:])

        def emit_L1(j):
            jj = j * 512
            w = min(512, R2 - jj)
            nch = w // 128
            s0 = (4 * j) % 8
            hj = 512 * (j % 2)
            for name, xsel, h1, w1s, _ in paths:
                psl = ps.tile([E, 512], f32, name=name + "_l1",
                              tag="l2", bufs=2)
                nc.tensor.matmul(psl[:, :w], lhsT=w1s[:],
                                 rhs=xt_v[:, s0:s0 + nch, xsel, :],
                                 start=True, stop=True)
                nc.scalar.activation(h1[0:E, hj:hj + w], psl[:, :w], AF.Relu)

        def emit_L2(t):
            for name, _, h1, _, w2bs in paths:
                segacc = segaccs[name]
                yab = ps.tile([128, NH], f32, name=name + "_l2",
                              tag="l2", bufs=2)
                lhsT = h1[:, 128 * (t % 8):128 * (t % 8 + 1)]
                nc.tensor.matmul(yab[:, 0:512], lhsT=lhsT,
                                 rhs=w2bs[:, 0:512], start=True, stop=True)
                nc.tensor.matmul(yab[:, 512:NH], lhsT=lhsT,
                                 rhs=w2bs[:, 512:NH], start=True, stop=True)
                y2 = y2p.tile([128, NH], bf16, name=name + "_y2", tag="y2",
                              bufs=2)
                if t >= NT - 6 and name == "qc":
                    nc.vector.tensor_scalar(y2[:], yab[:], 0.0, None,
                                            op0=ALU.max)
                else:
                    nc.scalar.activation(y2[:], yab[:], AF.Relu)
                if t < 32:
                    for f in range(8):
                        nc.tensor.matmul(
                            segacc[:, 64 * f + 2 * t:64 * f + 2 * t + 2],
                            lhsT=y2[:, 128 * f:128 * (f + 1)],
                            rhs=gmask_sb[:, 2 * t:2 * t + 2],
                            start=True, stop=True,
                        )
                else:
                    instacc = ps.tile([128, 512], f32, name=name + "_ins",
                                      tag="l2", bufs=2)
                    instaccs[name] = instacc
                    for f in range(8):
                        nc.tensor.matmul(
                            instacc[:, 64 * f:64 * (f + 1)],
                            lhsT=y2[:, 128 * f:128 * (f + 1)],
                            rhs=e64_sb[:],
                            start=True, stop=True,
                        )

        emit_gather(0)
        emit_gather(1)
        emit_gather(2)
        n_slab = 0
        for t in range(NT):
            if t % 4 == 1 and t >= 5:
                emit_L1((t - 5) // 4)
            if t >= 5:
                emit_L2(t - 5)
            emit_scores(t)
            # paced Q2-weight prefetch: gate on this chunk's gathered tile,
            # then ship slabs on the idle SP queue
            if 1 <= t <= 32:
                nc.sync.dma_start(zgate_d[:, t:t + 1],
                                  ec_tiles[min(t + 2, NT - 1)][0:1, 0:1])
                target = min(len(slab_jobs), (8 * t) // 5)
                while n_slab < target:
                    dst, srcap = slab_jobs[n_slab]
                    nc.sync.dma_start(dst, srcap)
                    n_slab += 1
            if t >= 1:
                emit_transposes(t - 1)
            if t + 3 < NT:
                emit_gather(t + 3)
            if t == 32:
                emit_L1(7)
        emit_transposes(NT - 1)
        while n_slab < len(slab_jobs):
            dst, srcap = slab_jobs[n_slab]
            nc.sync.dma_start(dst, srcap)
            n_slab += 1
        emit_L1(8)
        for t in range(NT - 5, NT):
            emit_L2(t)

        def build_x2(name):
            segacc, instacc = segaccs[name], instaccs[name]
            # q2 inputs: [embed | embed + inst-embed], feature-major blocks
            # hw allows only one PSUM operand per instruction: stage the
            # inst transpose in SBUF first
            iT = q2wk.tile([128, 512], bf16, name=name + "_iT")
            nc.scalar.copy(iT[:], instacc[:])
            x2 = q2wk.tile([128, NH], pdt[name], name=name + "_x2")
            x2v = x2[:].rearrange("p (f c) -> p f c", f=8)
            sv = segacc[:].rearrange("p (f c) -> p f c", f=8)
            iv = iT[:].rearrange("p (f c) -> p f c", f=8)
            nc.scalar.copy(x2v[:, :, 0:64], sv[:])
            nc.vector.tensor_tensor(x2v[:, :, 64:128], sv[:], iv[:],
                                    op=ALU.add)
            return x2

        x2_set = build_x2("qs")
        x2_ctx = build_x2("qc")

        # ---------------- phase D: Q2 MLPs
        def q2_mlp(x2, name, b1row, b2row, w3s, b3s, out_row):
            # generator-based so the two paths can be emitted interleaved;
            # 4 m-blocks share one [128,512] PSUM tile so PE runs ~1.7us
            # bursts per drain; biases enter as a K=1 matmul of ones
            w1t_sb = w1t_res[name]
            w2t_sb = w2t_res[name]
            hq = q2wk.tile([128, CH], pdt[name], name=name + "_hq")
            for mg in range(4):
                psq = ps.tile([128, 512], f32, name=name + "_p1",
                              tag="l2", bufs=2)
                for mi in range(4):
                    m = 4 * mg + mi
                    for k in range(8):
                        nc.tensor.matmul(
                            psq[:, 128 * mi:128 * (mi + 1)],
                            lhsT=w1t_sb[:, NH * m + 128 * k:
                                        NH * m + 128 * (k + 1)],
                            rhs=x2[:, 128 * k:128 * (k + 1)],
                            start=(k == 0), stop=False,
                        )
                    nc.tensor.matmul(
                        psq[:, 128 * mi:128 * (mi + 1)],
                        lhsT=b1row[:, 128 * m:128 * (m + 1)],
                        rhs=ones1[name][:], start=False, stop=True,
                    )
                hsc = 1.0
                if mg % 2 == 0:
                    nc.scalar.activation(hq[:, 512 * mg:512 * (mg + 1)],
                                         psq[:], AF.Relu, scale=hsc)
                else:
                    nc.vector.tensor_scalar(hq[:, 512 * mg:512 * (mg + 1)],
                                            psq[:], hsc, 0.0,
                                            op0=ALU.mult, op1=ALU.max)
                yield
            h2 = q2wk.tile([128, CH2], bf16, name=name + "_h2")
            for mg in range(2):
                psq = ps.tile([128, 512], f32, name=name + "_p2",
                              tag="l2", bufs=2)
                for mi in range(4):
                    m = 4 * mg + mi
                    for k in range(16):
                        nc.tensor.matmul(
                            psq[:, 128 * mi:128 * (mi + 1)],
                            lhsT=w2t_sb[:, CH * m + 128 * k:
                                        CH * m + 128 * (k + 1)],
                            rhs=hq[:, 128 * k:128 * (k + 1)],
                            start=(k == 0), stop=False,
                        )
                    nc.tensor.matmul(
                        psq[:, 128 * mi:128 * (mi + 1)],
                        lhsT=b2row[:, 128 * m:128 * (m + 1)],
                        rhs=ones1[name][:], start=False, stop=True,
                    )
                # undo the activation scaling (qs: 2048x, qc: 512x)
                usc = 1.0 / 256
                if mg % 2 == 0:
                    nc.vector.tensor_scalar(h2[:, 512 * mg:512 * (mg + 1)],
                                            psq[:], usc, 0.0,
                                            op0=ALU.mult, op1=ALU.max)
                else:
                    nc.scalar.activation(h2[:, 512 * mg:512 * (mg + 1)],
                                         psq[:], AF.Relu, scale=usc)
                yield
            ps3 = ps.tile([1, 128], f32, name=name + "_p3",
                          tag="xtps", bufs=2)
            for k in range(8):
                nc.tensor.matmul(
                    ps3[:],
                    lhsT=w3s[:, k:k + 1],
                    rhs=h2[:, 128 * k:128 * (k + 1)],
                    start=(k == 0), stop=(k == 7),
                )
            osb = q2wk.tile([1, 128], f32, name=name + "_o")
            nc.scalar.activation(osb[:], ps3[:], AF.Identity, bias=b3s[:])
            nc.sync.dma_start(out_row, osb[:])
            yield

        gens = [
            q2_mlp(x2_set, "qs", b1c_sb, b2c_sb, w3c_sb, b3_sb,
                   out_d[0:1, :]),
            q2_mlp(x2_ctx, "qc", b1ch_sb, b2ch_sb, w3ch_sb, b3h_sb,
                   out_d[1:2, :]),
        ]
        alive = list(gens)
        while alive:
            for g in list(alive):
                try:
                    next(g)
                    next(g)
                except StopIteration:
                    alive.remove(g)

    nc.compile()
    _CACHE[key] = nc
    return nc


# ---------------------------------------------------------------- host prep
def _q2_pretranspose(w, n_m, n_k, dt):
    # [p, m*(n_k*128) + k*128 + c] = w[128k+p, 128m+c]
    kdim, mdim = w.shape
    assert kdim == n_k * 128 and mdim == n_m * 128
    wt = w.reshape(n_k, 128, n_m, 128).transpose(1, 2, 0, 3)
    return np.ascontiguousarray(wt.reshape(128, n_m * n_k * 128)).astype(dt)


def _shared_consts():
    if "consts" in _CACHE:
        return _CACHE["consts"]
    c = {
        "e64": (np.eye(128, 64) * 256.0).astype(ml_dtypes.bfloat16),
        "i128": np.eye(128).astype(ml_dtypes.bfloat16),
        "i128f": np.eye(128, dtype=np.float32),
    }
    _CACHE["consts"] = c
    return c


def make_in_maps(inputs):
    """inputs: dict of FULL numpy arrays keyed as in setup_inputs()."""
    inp = {k: np.asarray(v) for k, v in inputs.items()}
    set_ids = inp["set_ids"].astype(np.int32)
    inst_ids = inp["inst_ids"].astype(np.int32)
    ca32 = np.ascontiguousarray(inp["contex_array"].astype(np.int32))
    emb = np.ascontiguousarray(inp["emb"].astype(np.float32))

    embj = np.empty((V, (C + 1) * E), np.float32)
    embj[:, :E] = emb
    for c in range(C):
        embj[:, (c + 1) * E:(c + 2) * E] = emb[ca32[:, c], :]
    bf = ml_dtypes.bfloat16
    f8 = ml_dtypes.float8_e4m3
    shared = {
        "embj": embj,
        "w1": np.ascontiguousarray(inp["q1_w1"].astype(np.float32)).astype(bf),
        "w2b": np.ascontiguousarray(
            np.vstack([inp["q1_w2"], inp["q1_b2"][None, :]])
        ).astype(bf),
        "w1h": np.ascontiguousarray(
            inp["q1h_w1"].astype(np.float32)).astype(bf),
        "w2bh": np.ascontiguousarray(
            np.vstack([inp["q1h_w2"], inp["q1h_b2"][None, :]])
        ).astype(bf),
        "q2w1t": _q2_pretranspose(np.asarray(inp["q2_w1"], np.float32),
                                  16, 8, bf),
        "q2w2t": _q2_pretranspose(np.asarray(inp["q2_w2"], np.float32),
                                  8, 16, bf),
        "q2hw1t": _q2_pretranspose(np.asarray(inp["q2h_w1"], np.float32),
                                   16, 8, bf),
        "q2hw2t": _q2_pretranspose(np.asarray(inp["q2h_w2"], np.float32),
                                   8, 16, bf),
        "b1c": (inp["q2_b1"].astype(np.float32) * 256).reshape(1, CH)
            .astype(bf),
        "b2c": (inp["q2_b2"].astype(np.float32) * 256).reshape(1, CH2)
            .astype(bf),
        "w3c": np.ascontiguousarray(
            inp["q2_w3"].astype(np.float32).reshape(8, 128).T).astype(bf),
        "b3": inp["q2_b3"].astype(np.float32).reshape(1, 1),
        "b1ch": (inp["q2h_b1"].astype(np.float32) * 256).reshape(1, CH)
            .astype(bf),
        "b2ch": (inp["q2h_b2"].astype(np.float32) * 256).reshape(1, CH2)
            .astype(bf),
        "w3ch": np.ascontiguousarray(
            inp["q2h_w3"].astype(np.float32).reshape(8, 128).T).astype(bf),
        "b3h": inp["q2h_b3"].astype(np.float32).reshape(1, 1),
    }
    shared.update(_shared_consts())

    in_maps = []
    for c in range(NCORES):
        sid = set_ids[c * BC:(c + 1) * BC]          # [64, 64]
        iid = inst_ids[c * BC:(c + 1) * BC, 0]      # [64]
        ids_flat = np.concatenate(
            [sid.reshape(-1), iid,
             np.ones(R2 - R_REAL, np.int32)]).astype(np.int32)
        ids_ch = np.ascontiguousarray(ids_flat.reshape(NT, 128).T)
        mask = (sid != 0).astype(np.float32) * 256.0  # [64, 64]; x256 puts
        # the fp8-cast Q2 inputs in e4m3's normal range (e4m3 max is 240)
        gmask = np.zeros((128, 64), np.float32)  # cast to bf16 below
        for t in range(32):
            gmask[0:64, 2 * t] = mask[2 * t, :]
            gmask[64:128, 2 * t + 1] = mask[2 * t + 1, :]
        m = dict(shared)
        m["ids_ch"] = ids_ch
        m["gmask"] = gmask.astype(ml_dtypes.bfloat16)
        in_maps.append(m)
    return in_maps


def assemble_outputs(results):
    """results: list (per core) of dicts with 'out' [2, 128]."""
    setQ2 = np.zeros((B, 1), np.float32)
    setInst = np.zeros((B, 1), np.float32)
    ctxHat = np.zeros((B, 1), np.float32)
    ctxInstHat = np.zeros((B, 1), np.float32)
    for c in range(NCORES):
        o = np.asarray(results[c]["out"])
        setQ2[c * BC:(c + 1) * BC, 0] = o[0, 0:BC]
        setInst[c * BC:(c + 1) * BC, 0] = o[0, BC:2 * BC]
        ctxHat[c * BC:(c + 1) * BC, 0] = o[1, 0:BC]
        ctxInstHat[c * BC:(c + 1) * BC, 0] = o[1, BC:2 * BC]
    return (setQ2, setInst, ctxHat, ctxInstHat)


def run_cores(inputs, trace=False, **kw):
    nc = build_program()
    in_maps = make_in_maps(inputs)
    res = run_bass_kernel_spmd(nc, in_maps, list(range(NCORES)),
                               trace=trace, **kw)
    return assemble_outputs(res.results), res


def kernel(**inputs):
    outs, _ = run_cores(inputs, trace=False)
    return outs
